# revision 11
# baseline (speedup 1.0000x reference)
"""GCN (3x ChebConv K=3 + global mean pool + linear head) on 8 Trainium2
NeuronCores via Bass/Tile — matmul-scatter design with split-half
AllGather pipelining.

Per layer (fin -> fout, weights W[0..2]):
    out = H (W0 - W2) + L (H W1 + 2 L (H W2)),   L = -D^-1/2 A D^-1/2
Both L applications are gather + B-matrix matmul-scatter:
  - edges dst-partitioned across 8 cores, grouped per 128-row dst window
    into fixed-count 128-edge subgroups (max over cores -> SPMD-invariant),
  - gather src rows from replicated bf16 tables via gpsimd dma_gather
    (>=256B rows), scatter via PE matmul with a DVE-built selection matrix
    B[e, r] = ea_e * (dstloc_e == r) accumulating in PSUM per window,
  - L(H W2) = (L H) W2: the first L gathers the H table itself and applies
    W2 per window after the scatter, so no intermediate U table exists.
Every replicated table is split in two by source-row half (A: loc<3200,
B: loc>=3200), each half AllGathered separately the moment its windows
are produced; each propagate consumes in two passes (A-subgroups into a
bf16 partial slab, then B-subgroups + combine), so half of each gather
pass overlaps the other half's AllGather. Narrow tables (<=64 cols) pack
two logical rows per 256B row (gather idx g//2, two parity-masked B
matmuls per subgroup). Dense per-window matmuls are precomputed into bf16
slabs overlapping the collectives. dinv comes from a host-packed ea slot
layout reduced on DVE.
"""
import sys
sys.path.insert(0, "/opt/trn_rl_repo")
import numpy as np

P = 128
NCORES = 8
N, E, FIN, NG = 50000, 500000, 160, 128
RPC = N // NCORES            # 6250
NB = (RPC + P - 1) // P      # 49
RB = NB * P                  # 6272
NTOT = RB * NCORES           # 50176
F1, F2, F3 = 128, 64, 32
XW = 256                     # x~ table cols (bf16; 160 real)
TW = 128                     # wide table cols (bf16)
NBA = 25                     # windows in half A
NBB = NB - NBA               # 24
LOCA = NBA * P               # 3200 rows/core in half A
LOCB = NBB * P               # 3072
NRA, NRB = NCORES * LOCA, NCORES * LOCB    # 25600, 24576 (< 32768)
CALL_SG_X = 32               # subgroups per gather call, 512B rows
CALL_SG_T = 64               # subgroups per gather call, 256B rows


def _wrap16(flat):
    w = np.ascontiguousarray(flat.reshape(-1, 16).T).astype(np.int16)
    return np.tile(w, (8, 1))


def _prep(inputs):
    x = np.asarray(inputs["x"], np.float32)
    ei = np.asarray(inputs["edge_index"]).astype(np.int64)
    ea = np.asarray(inputs["edge_attr"], np.float32)
    batch = np.asarray(inputs["batch"]).astype(np.int64)
    Ws = [np.asarray(inputs[k], np.float32) for k in ("W1", "W2", "W3")]
    bs = [np.asarray(inputs[k], np.float32) for k in ("b1", "b2", "b3")]
    Wl = np.asarray(inputs["Wl"], np.float32)
    bl = np.asarray(inputs["bl"], np.float32)

    src, dst = ei[0], ei[1]
    src_core = src // RPC
    srcloc = src % RPC
    inA = srcloc < LOCA
    gA = src_core * LOCA + srcloc              # row in half-A tables
    gB = src_core * LOCB + (srcloc - LOCA)     # row in half-B tables
    owner = dst // RPC
    dstloc = dst % RPC
    wid = dstloc // P
    wloc = dstloc % P

    # --- per (core, window) A/B edge lists ---
    lists = [[None] * NB for _ in range(NCORES)]
    for c in range(NCORES):
        mc = np.nonzero(owner == c)[0]
        wsub = wid[mc]
        order = np.argsort(wsub, kind="stable")
        mc = mc[order]
        bounds = np.searchsorted(wsub[order], np.arange(NB + 1))
        for w in range(NB):
            m = mc[bounds[w]:bounds[w + 1]]
            ma = m[inA[m]]
            mb = m[~inA[m]]
            ma = ma[np.argsort(gA[ma], kind="stable")]
            mb = mb[np.argsort(gB[mb], kind="stable")]
            lists[c][w] = (ma, mb)

    S_A = [max(-(-len(lists[c][w][0]) // P) for c in range(NCORES))
           for w in range(NB)]
    S_B = [max(-(-len(lists[c][w][1]) // P) for c in range(NCORES))
           for w in range(NB)]
    NSUBA, NSUBB = sum(S_A), sum(S_B)
    NSUB = NSUBA + NSUBB
    a_pre = np.concatenate([[0], np.cumsum(S_A)]).astype(int)
    b_pre = np.concatenate([[0], np.cumsum(S_B)]).astype(int)

    # schedule: per window, subgroup q lists per stream
    schedA = [[a_pre[w] + j for j in range(S_A[w])] for w in range(NB)]
    schedB = [[b_pre[w] + j for j in range(S_B[w])] for w in range(NB)]

    # --- deg slot layout (own src rows) ---
    cnts = np.zeros((NCORES, RPC), np.int64)
    for c in range(NCORES):
        cnts[c] = np.bincount(srcloc[src_core == c], minlength=RPC)
    K_DEG = int(cnts.max())

    # --- replicated x tables (A/B row spaces, padded rows zero) ---
    xfullA = np.zeros((NRA, FIN), np.float32)
    xfullB = np.zeros((NRB, FIN), np.float32)
    for c in range(NCORES):
        xa = x[c * RPC:c * RPC + LOCA]
        xfullA[c * LOCA:c * LOCA + len(xa)] = xa
        xb = x[c * RPC + LOCA:(c + 1) * RPC]
        xfullB[c * LOCB:c * LOCB + len(xb)] = xb

    iota_f = np.tile(np.arange(P, dtype=np.float32), (P, 1))
    ident_f = np.eye(P, dtype=np.float32)
    import ml_dtypes
    bf16 = ml_dtypes.bfloat16

    in_maps = []
    for c in range(NCORES):
        A_idx = np.zeros(max(NSUBA, 1) * P, np.int64)
        B_idx = np.zeros(max(NSUBB, 1) * P, np.int64)
        Ap_idx = np.zeros(max(NSUBA, 1) * P, np.int64)
        Bp_idx = np.zeros(max(NSUBB, 1) * P, np.int64)
        bcol = np.zeros((P, NSUB), np.float32)
        eacol = np.zeros((P, NSUB), np.float32)
        eacolA = np.zeros((P, NSUB), np.float32)
        eacolB = np.zeros((P, NSUB), np.float32)

        def fill(m, col, gg, idx_arr, pidx_arr, q):
            n = len(m)
            idx_arr[q * P:q * P + n] = gg[m]
            pidx_arr[q * P:q * P + n] = gg[m] // 2
            bcol[:n, col] = wloc[m]
            eacol[:n, col] = ea[m]
            par = (gg[m] % 2).astype(np.float32)
            eacolA[:n, col] = ea[m] * (1.0 - par)
            eacolB[:n, col] = ea[m] * par

        for w in range(NB):
            ma, mb = lists[c][w]
            for j in range(S_A[w]):
                q = a_pre[w] + j
                fill(ma[j * P:(j + 1) * P], q, gA, A_idx, Ap_idx, q)
            for j in range(S_B[w]):
                q = b_pre[w] + j
                fill(mb[j * P:(j + 1) * P], NSUBA + q, gB, B_idx, Bp_idx, q)

        dslot = np.zeros((P, NB * K_DEG), np.float32)
        me = np.nonzero(src_core == c)[0]
        slot_ctr = np.zeros(RPC, np.int64)
        locs = srcloc[me]
        for e, loc in zip(me, locs):
            s = slot_ctr[loc]
            slot_ctr[loc] += 1
            dslot[loc % P, (loc // P) * K_DEG + s] = ea[e]

        batchc = np.full((P, NB), 999.0, np.float32)
        blk = np.full(RB, 999.0, np.float32)
        blk[:RPC] = batch[c * RPC:(c + 1) * RPC]
        batchc[:, :] = blk.reshape(NB, P).T

        xT_own = np.zeros((FIN, RB), np.float32)
        xT_own[:, :RPC] = x[c * RPC:(c + 1) * RPC].T

        im = dict(
            xfullA=xfullA, xfullB=xfullB, xT=xT_own,
            gia=_wrap16(A_idx), gib=_wrap16(B_idx),
            gpa=_wrap16(Ap_idx), gpb=_wrap16(Bp_idx),
            bcol=bcol, eacol=eacol, eacolA=eacolA, eacolB=eacolB,
            dslot=dslot, batchc=batchc,
            w1=Ws[0], w2=Ws[1], w3=Ws[2], wl=Wl,
            b1=bs[0][None, :], b2=bs[1][None, :], b3=bs[2][None, :],
            blb=np.tile(bl, (P, 1)),
            iotaf=iota_f, iotab=iota_f.astype(bf16),
            identb=ident_f.astype(bf16), identf=ident_f,
        )
        in_maps.append(im)

    hp = dict(NSUBA=NSUBA, NSUBB=NSUBB, NSUB=NSUB, K_DEG=K_DEG,
              schedA=schedA, schedB=schedB)
    return hp, in_maps


def _build(hp):
    import concourse.bacc as bacc
    import concourse.tile as tile
    import concourse.mybir as mybir
    from concourse import library_config
    dt = mybir.dt
    AF = mybir.ActivationFunctionType
    OP = mybir.AluOpType
    f32, bf = dt.float32, dt.bfloat16

    NSUBA, NSUBB, NSUB = hp["NSUBA"], hp["NSUBB"], hp["NSUB"]
    K_DEG = hp["K_DEG"]
    schedA, schedB = hp["schedA"], hp["schedB"]

    nc = bacc.Bacc("TRN2", target_bir_lowering=False, debug=False,
                   num_devices=NCORES, dynamic_dma_scratch_size=24576)

    xfullA = nc.dram_tensor("xfullA", [NRA, FIN], f32, kind="ExternalInput")
    xfullB = nc.dram_tensor("xfullB", [NRB, FIN], f32, kind="ExternalInput")
    xT = nc.dram_tensor("xT", [FIN, RB], f32, kind="ExternalInput")
    gia = nc.dram_tensor("gia", [P, max(NSUBA, 1) * 8], dt.int16,
                         kind="ExternalInput")
    gib = nc.dram_tensor("gib", [P, max(NSUBB, 1) * 8], dt.int16,
                         kind="ExternalInput")
    gpa = nc.dram_tensor("gpa", [P, max(NSUBA, 1) * 8], dt.int16,
                         kind="ExternalInput")
    gpb = nc.dram_tensor("gpb", [P, max(NSUBB, 1) * 8], dt.int16,
                         kind="ExternalInput")
    bcol = nc.dram_tensor("bcol", [P, NSUB], f32, kind="ExternalInput")
    eacol = nc.dram_tensor("eacol", [P, NSUB], f32, kind="ExternalInput")
    eacolA = nc.dram_tensor("eacolA", [P, NSUB], f32, kind="ExternalInput")
    eacolB = nc.dram_tensor("eacolB", [P, NSUB], f32, kind="ExternalInput")
    dslot = nc.dram_tensor("dslot", [P, NB * K_DEG], f32,
                           kind="ExternalInput")
    batchc = nc.dram_tensor("batchc", [P, NB], f32, kind="ExternalInput")
    w1 = nc.dram_tensor("w1", [3, FIN, F1], f32, kind="ExternalInput")
    w2 = nc.dram_tensor("w2", [3, F1, F2], f32, kind="ExternalInput")
    w3 = nc.dram_tensor("w3", [3, F2, F3], f32, kind="ExternalInput")
    wl = nc.dram_tensor("wl", [F3, 2], f32, kind="ExternalInput")
    b1 = nc.dram_tensor("b1", [1, F1], f32, kind="ExternalInput")
    b2 = nc.dram_tensor("b2", [1, F2], f32, kind="ExternalInput")
    b3 = nc.dram_tensor("b3", [1, F3], f32, kind="ExternalInput")
    blb = nc.dram_tensor("blb", [P, 2], f32, kind="ExternalInput")
    iotaf = nc.dram_tensor("iotaf", [P, P], f32, kind="ExternalInput")
    iotab = nc.dram_tensor("iotab", [P, P], bf, kind="ExternalInput")
    identb = nc.dram_tensor("identb", [P, P], bf, kind="ExternalInput")
    identf = nc.dram_tensor("identf", [P, P], f32, kind="ExternalInput")
    y = nc.dram_tensor("y", [P, 2], f32, kind="ExternalOutput")

    with tile.TileContext(nc) as tc:
        with tc.tile_pool(name="cst", bufs=1) as cst, \
             tc.tile_pool(name="wk", bufs=3) as wk, \
             tc.tile_pool(name="wk1", bufs=1) as wk1, \
             tc.tile_pool(name="bp", bufs=4) as bp, \
             tc.tile_pool(name="slb", bufs=1) as slb, \
             tc.tile_pool(name="vlo", bufs=2) as vlo, \
             tc.tile_pool(name="vhi", bufs=2) as vhi, \
             tc.tile_pool(name="ixp", bufs=2) as ixp, \
             tc.tile_pool(name="psm", bufs=2, space="PSUM") as psm, \
             tc.tile_pool(name="psz", bufs=2, space="PSUM") as psz, \
             tc.tile_pool(name="pstr", bufs=1, space="PSUM") as pstr, \
             tc.tile_pool(name="psfin", bufs=1, space="PSUM") as psfin, \
             tc.tile_pool(name="dram", bufs=1, space="DRAM") as dram:

            nc.gpsimd.load_library(library_config.mlp)

            # ---------------- dram tables ----------------
            def shared_pair(tag, rows_w):
                ta = dram.tile([NCORES * rows_w[0], rows_w[2]], bf,
                               tag=tag + "A", name=tag + "A",
                               addr_space="Shared")
                tb = dram.tile([NCORES * rows_w[1], rows_w[2]], bf,
                               tag=tag + "B", name=tag + "B",
                               addr_space="Shared")
                return ta, tb

            TxA = dram.tile([NRA, XW], bf, tag="TxA", name="TxA")
            TxB = dram.tile([NRB, XW], bf, tag="TxB", name="TxB")
            Tz1A, Tz1B = shared_pair("Tz1", (LOCA, LOCB, TW))
            Th1A, Th1B = shared_pair("Th1", (LOCA, LOCB, TW))
            Tz2A, Tz2B = shared_pair("Tz2", (LOCA // 2, LOCB // 2, TW))
            Th2A, Th2B = shared_pair("Th2", (LOCA // 2, LOCB // 2, TW))
            Tz3A, Tz3B = shared_pair("Tz3", (LOCA // 2, LOCB // 2, TW))
            zcon = dram.tile([RB, TW], bf, tag="zcon", name="zcon")
            zcon2 = dram.tile([RB, TW // 2], bf, tag="zcon2", name="zcon2")
            hcon = dram.tile([RB, TW], bf, tag="hcon", name="hcon")
            hcon2 = dram.tile([RB, TW // 2], bf, tag="hcon2", name="hcon2")
            hown = dram.tile([RB, TW], bf, tag="hown", name="hown")
            xTbf = dram.tile([FIN, RB], bf, tag="xTbf", name="xTbf")
            degsh = dram.tile([RB, 1], f32, tag="degsh", name="degsh")
            degf = dram.tile([NTOT, 1], f32, tag="degf", name="degf")
            arin = dram.tile([P, F3 + 1], f32, tag="arin", name="arin")
            arout = dram.tile([P, F3 + 1], f32, tag="arout", name="arout")

            # ---------------- consts ----------------
            iotab_t = cst.tile([P, P], bf)
            nc.sync.dma_start(out=iotab_t[:], in_=iotab[:, :])
            iotaf_t = cst.tile([P, P], f32)
            nc.sync.dma_start(out=iotaf_t[:], in_=iotaf[:, :])
            identb_t = cst.tile([P, P], bf)
            nc.sync.dma_start(out=identb_t[:], in_=identb[:, :])
            identf_t = cst.tile([P, P], f32)
            nc.sync.dma_start(out=identf_t[:], in_=identf[:, :])
            bcol_t = cst.tile([P, NSUB], f32)
            nc.sync.dma_start(out=bcol_t[:], in_=bcol[:, :])
            eacol_t = cst.tile([P, NSUB], f32)
            nc.sync.dma_start(out=eacol_t[:], in_=eacol[:, :])
            eaA_t = cst.tile([P, NSUB], f32)
            nc.sync.dma_start(out=eaA_t[:], in_=eacolA[:, :])
            eaB_t = cst.tile([P, NSUB], f32)
            nc.sync.dma_start(out=eaB_t[:], in_=eacolB[:, :])
            batchc_t = cst.tile([P, NB], f32)
            nc.sync.dma_start(out=batchc_t[:], in_=batchc[:, :])
            ones1 = cst.tile([1, P], f32)
            nc.vector.memset(ones1[:], 1.0)
            blt = cst.tile([P, 2], f32)
            nc.sync.dma_start(out=blt[:], in_=blb[:, :])
            wlt = cst.tile([P, 2], f32)
            nc.sync.dma_start(out=wlt[:F3, :], in_=wl[:, :])
            b_t = []
            for bb, fo in ((b1, F1), (b2, F2), (b3, F3)):
                t = cst.tile([1, fo], f32, tag=f"b{fo}")
                nc.sync.dma_start(out=t[:], in_=bb[:, :])
                b_t.append(t)

            # weights -> bf16 chunk tiles: Wa = W0 - W2, Wb = W1, Wc = W2
            layer_w = []
            for li, (wt_, fin, fo) in enumerate(
                    ((w1, FIN, F1), (w2, F1, F2), (w3, F2, F3))):
                nch = (fin + P - 1) // P
                was, wbs, wcs = [], [], []
                for o in range(nch):
                    kp = min(P, fin - o * P)
                    t0 = wk.tile([P, fo], f32, tag="wld", bufs=2)
                    nc.sync.dma_start(out=t0[:kp, :],
                                      in_=wt_[0, o * P:o * P + kp, :])
                    t2 = wk.tile([P, fo], f32, tag="wld", bufs=2)
                    nc.sync.dma_start(out=t2[:kp, :],
                                      in_=wt_[2, o * P:o * P + kp, :])
                    t1 = wk.tile([P, fo], f32, tag="wld", bufs=2)
                    nc.sync.dma_start(out=t1[:kp, :],
                                      in_=wt_[1, o * P:o * P + kp, :])
                    wa = cst.tile([P, fo], bf, tag=f"wa{li}_{o}")
                    nc.vector.tensor_tensor(out=wa[:kp, :], in0=t0[:kp, :],
                                            in1=t2[:kp, :], op=OP.subtract)
                    wb_ = cst.tile([P, fo], bf, tag=f"wb{li}_{o}")
                    nc.vector.tensor_copy(out=wb_[:kp, :], in_=t1[:kp, :])
                    wc_ = cst.tile([P, fo], bf, tag=f"wc{li}_{o}")
                    nc.vector.tensor_copy(out=wc_[:kp, :], in_=t2[:kp, :])
                    was.append((wa, kp))
                    wbs.append((wb_, kp))
                    wcs.append((wc_, kp))
                layer_w.append(dict(wa=was, wb=wbs, wc=wcs))

            # xT -> bf16 DRAM copy (lhsT source for l=0 dense mms)
            XCH = [(0, P), (1, FIN - P)]
            for o, st_tag in ((0, "hslab"), (1, "htslab")):
                kp = min(P, FIN - o * P)
                xstage = slb.tile([P, RB], bf, tag=st_tag,
                                  name=f"xstage{o}")
                nc.gpsimd.dma_start(out=xstage[:kp, :],
                                    in_=xT[o * P:o * P + kp, :])
                nc.sync.dma_start(out=xTbf[o * P:o * P + kp, :],
                                  in_=xstage[:kp, :])

            # ---------------- deg / dinv ----------------
            degsb = wk1.tile([P, NB], f32, tag="degsb")
            CH_D = 7
            for c0 in range(0, NB, CH_D):
                ch = min(CH_D, NB - c0)
                t = wk.tile([P, CH_D * K_DEG], f32, tag="dgl", bufs=2)
                nc.sync.dma_start(
                    out=t[:, :ch * K_DEG],
                    in_=dslot[:, c0 * K_DEG:(c0 + ch) * K_DEG])
                nc.vector.tensor_reduce(
                    out=degsb[:, c0:c0 + ch, None],
                    in_=t[:, :ch * K_DEG].rearrange("p (b k) -> p b k",
                                                    k=K_DEG),
                    axis=mybir.AxisListType.X, op=OP.add)
            nc.sync.dma_start(
                out=degsh[:].rearrange("(b p) c -> p (b c)", p=P),
                in_=degsb[:])
            nc.gpsimd.collective_compute(
                "AllGather", OP.bypass, replica_groups=[list(range(NCORES))],
                ins=[degsh[:, :].opt()], outs=[degf[:, :].opt()])

            def dinv_of(deg_ap, cols, tag):
                m = wk1.tile([P, cols], f32, tag=tag + "m")
                nc.vector.tensor_scalar(out=m[:], in0=deg_ap, scalar1=0.0,
                                        scalar2=None, op0=OP.is_le)
                safe = wk1.tile([P, cols], f32, tag=tag + "s")
                nc.vector.tensor_tensor(out=safe[:], in0=deg_ap, in1=m[:],
                                        op=OP.add)
                sq = wk1.tile([P, cols], f32, tag=tag + "q")
                nc.scalar.activation(out=sq[:], in_=safe[:], func=AF.Sqrt)
                rcp = wk1.tile([P, cols], f32, tag=tag + "r")
                nc.vector.reciprocal(rcp[:], sq[:])
                gm = wk1.tile([P, cols], f32, tag=tag + "g")
                nc.vector.tensor_scalar(out=gm[:], in0=deg_ap, scalar1=0.0,
                                        scalar2=None, op0=OP.is_gt)
                dv = cst.tile([P, cols], f32, tag=tag + "d")
                nc.vector.tensor_tensor(out=dv[:], in0=rcp[:], in1=gm[:],
                                        op=OP.mult)
                return dv

            dinv_own = dinv_of(degsb[:], NB, "dow")
            negd_own = cst.tile([P, NB], f32)
            nc.vector.tensor_scalar_mul(negd_own[:], dinv_own[:], -1.0)
            d2 = wk1.tile([P, NB], f32, tag="d2")
            nc.vector.tensor_tensor(out=d2[:], in0=dinv_own[:],
                                    in1=dinv_own[:], op=OP.mult)
            neg2d2_own = cst.tile([P, NB], f32)
            nc.vector.tensor_scalar_mul(neg2d2_own[:], d2[:], -2.0)

            NCOLT = NTOT // P
            degfsb = wk1.tile([P, NCOLT], f32, tag="degfsb")
            nc.sync.dma_start(
                out=degfsb[:],
                in_=degf[:, 0:1].rearrange("(b p) c -> p (b c)", p=P))
            dinv_full = dinv_of(degfsb[:], NCOLT, "dfu")

            # ---------------- x~ table builds (A then B) ----------------
            # half-A tile t -> old slab col (t//NBA)*NB + t%NBA
            for (xf, tx, nhalf, nper, woff) in (
                    (xfullA, TxA, NRA // P, NBA, 0),
                    (xfullB, TxB, NRB // P, NBB, NBA)):
                xf_v = xf[:, :].rearrange("(t p) f -> p t f", p=P)
                tx_v = tx[:].rearrange("(t p) f -> p t f", p=P)
                CH_X = 10
                for t0 in range(0, nhalf, CH_X):
                    ch = min(CH_X, nhalf - t0)
                    xt_ = wk.tile([P, CH_X, FIN], f32, tag="xld", bufs=2)
                    nc.sync.dma_start(out=xt_[:, :ch, :],
                                      in_=xf_v[:, t0:t0 + ch, :])
                    xs = wk.tile([P, CH_X, FIN], bf, tag="xsc", bufs=2)
                    for t in range(ch):
                        ta = t0 + t
                        oldcol = (ta // nper) * NB + woff + ta % nper
                        nc.scalar.activation(
                            out=xs[:, t, :], in_=xt_[:, t, :], func=AF.Copy,
                            scale=dinv_full[:, oldcol:oldcol + 1])
                    nc.sync.dma_start(out=tx_v[:, t0:t0 + ch, 0:FIN],
                                      in_=xs[:, :ch, :])

            # ---------------- propagate machinery ----------------
            def gather_pass(sub_sched, gt_dram, goff_sub, nsub_s, tap,
                            pool, twidth, realw, call_sg, paired, fin_cb):
                """one half-pass: windows consume this stream's subgroups;
                fin_cb(w, psm_t_or_None) after each window's subgroups."""
                st = dict(issued=0, tiles=[])
                ncalls = -(-nsub_s // call_sg) if nsub_s else 0

                def issue():
                    a = st["issued"] * call_sg
                    b = min(a + call_sg, nsub_s)
                    nsg = b - a
                    it = ixp.tile([P, call_sg * 8], dt.int16, tag="it",
                                  name="it")
                    nc.sync.dma_start(
                        out=it[:, :nsg * 8],
                        in_=gt_dram[:, (goff_sub + a) * 8:
                                    (goff_sub + b) * 8])
                    vt = pool.tile([P, call_sg, twidth], bf,
                                   tag="vt", name="vt")
                    nc.gpsimd.dma_gather(
                        out_ap=vt[:, :nsg, :], in_ap=tap,
                        idxs_ap=it[:, :nsg * 8],
                        num_idxs=nsg * P, num_idxs_reg=nsg * P,
                        elem_size=twidth, single_packet=False)
                    st["tiles"].append((vt, a))
                    st["issued"] += 1

                def get(q):
                    while st["issued"] * call_sg <= q:
                        issue()
                    if st["issued"] < ncalls and \
                            q >= (st["issued"] - 1) * call_sg + call_sg // 2:
                        issue()
                    ci = q // call_sg
                    vt, a = st["tiles"][ci]
                    return vt, q - a

                for w in range(NB):
                    subs = sub_sched[w]
                    psm_t = None
                    if subs:
                        psm_t = psm.tile([P, realw], f32, tag="psm",
                                         name="psm_t")
                        nmm = len(subs) * (2 if paired else 1)
                        i = 0
                        for q in subs:
                            col = q if sub_sched is schedA else NSUBA + q
                            vt, slot = get(q)
                            if paired:
                                for eat, off in ((eaA_t, 0), (eaB_t, 64)):
                                    B = bp.tile([P, P], bf, tag="B",
                                                name="B")
                                    nc.vector.tensor_scalar(
                                        out=B[:], in0=iotab_t[:],
                                        scalar1=bcol_t[:, col:col + 1],
                                        scalar2=eat[:, col:col + 1],
                                        op0=OP.is_equal, op1=OP.mult)
                                    nc.tensor.matmul(
                                        psm_t[:], B[:],
                                        vt[:, slot, off:off + realw],
                                        start=(i == 0), stop=(i == nmm - 1))
                                    i += 1
                            else:
                                B = bp.tile([P, P], bf, tag="B", name="B")
                                nc.vector.tensor_scalar(
                                    out=B[:], in0=iotab_t[:],
                                    scalar1=bcol_t[:, col:col + 1],
                                    scalar2=eacol_t[:, col:col + 1],
                                    op0=OP.is_equal, op1=OP.mult)
                                nc.tensor.matmul(
                                    psm_t[:], B[:], vt[:, slot, 0:realw],
                                    start=(i == 0), stop=(i == nmm - 1))
                                i += 1
                    fin_cb(w, psm_t)

            def run_propagate(tapA, tapB, gtA, gtB, twidth, realw, call_sg,
                              paired, win_fn, half_cb=None):
                """pass A: scatter A-subgroups into mslab; pass B: combine
                with B-subgroups and call win_fn(w, m_ap) where m_ap is a
                f32-valued AP of the full scatter result (PSUM or SBUF).
                half_cb(half) is invoked after windows [0:NBA) / [NBA:NB)
                of pass B complete (for early contribution writes)."""
                mslab = slb.tile([P, NB, realw], bf, tag="mslab",
                                 name="mslab")
                has_a = [False] * NB

                def finA(w, psm_t):
                    if psm_t is not None:
                        has_a[w] = True
                        nc.scalar.activation(out=mslab[:, w, :],
                                             in_=psm_t[:], func=AF.Copy)

                gather_pass(schedA, gtA, 0, NSUBA, tapA, vlo, twidth,
                            realw, call_sg, paired, finA)

                def finB(w, psm_t):
                    if psm_t is not None and has_a[w]:
                        s = wk.tile([P, realw], f32, tag="msum")
                        nc.vector.tensor_tensor(
                            out=s[:], in0=psm_t[:], in1=mslab[:, w, :],
                            op=OP.add)
                        win_fn(w, s[:])
                    elif psm_t is not None:
                        win_fn(w, psm_t[:])
                    elif has_a[w]:
                        win_fn(w, mslab[:, w, :])
                    else:
                        win_fn(w, None)
                    if half_cb is not None:
                        if w == NBA - 1:
                            half_cb(0)
                        elif w == NB - 1:
                            half_cb(1)

                gather_pass(schedB, gtB, 0, NSUBB, tapB, vhi, twidth,
                            realw, call_sg, paired, finB)

            # ---------------- layers ----------------
            layer_cfg = [
                dict(fin=FIN, fout=F1, p1_paired=False, p2_paired=False,
                     tinA=TxA, tinB=TxB, tin_w=XW,
                     tzA=Tz1A, tzB=Tz1B, thA=Th1A, thB=Th1B),
                dict(fin=F1, fout=F2, p1_paired=False, p2_paired=True,
                     tinA=Th1A, tinB=Th1B, tin_w=TW,
                     tzA=Tz2A, tzB=Tz2B, thA=Th2A, thB=Th2B),
                dict(fin=F2, fout=F3, p1_paired=True, p2_paired=True,
                     tinA=Th2A, tinB=Th2B, tin_w=TW,
                     tzA=Tz3A, tzB=Tz3B, thA=None, thB=None),
            ]

            for li, cfg in enumerate(layer_cfg):
                fin, fout = cfg["fin"], cfg["fout"]
                lw = layer_w[li]
                nch_in = (fin + P - 1) // P
                p1_paired, p2_paired = cfg["p1_paired"], cfg["p2_paired"]

                # ---- P1 pre-pass: zpre[w] = (H~ W1)[own w] ----
                zpre = slb.tile([P, NB, F1], bf, tag="pre", name="zpre")
                for w in range(NB):
                    psz1 = psz.tile([P, fout], f32, tag="psz", name="psz1")
                    if li == 0:
                        for o, kp in XCH:
                            xw = wk.tile([P, P], bf, tag="xw")
                            nc.sync.dma_start(
                                out=xw[:kp, :],
                                in_=xTbf[o * P:o * P + kp,
                                         w * P:(w + 1) * P])
                            nc.tensor.matmul(
                                psz1[:], xw[:kp, :], lw["wb"][o][0][:kp, :],
                                start=(o == 0), stop=(o == len(XCH) - 1))
                    else:
                        for o in range(nch_in):
                            ht = wk.tile([P, P], bf, tag="htT")
                            nc.sync.dma_start(
                                out=ht[:],
                                in_=hcon[w * P:(w + 1) * P,
                                         o * P:(o + 1) * P],
                                transpose=True)
                            kp = lw["wb"][o][1]
                            nc.tensor.matmul(
                                psz1[:], ht[:kp, :], lw["wb"][o][0][:kp, :],
                                start=(o == 0), stop=(o == nch_in - 1))
                    if li == 0:
                        nc.scalar.activation(
                            out=zpre[:, w, 0:fout], in_=psz1[:],
                            func=AF.Copy, scale=dinv_own[:, w:w + 1])
                    else:
                        nc.scalar.activation(out=zpre[:, w, 0:fout],
                                             in_=psz1[:], func=AF.Copy)

                # ---- P1: M1 -> Z~ own (with early half contributions) ----
                zslab = slb.tile([P, NB, F1], bf, tag="zslab", name="zslab")

                def p1_win(w, m_ap, lw=lw, fin=fin, fout=fout,
                           nch_in=nch_in, zpre=zpre, zslab=zslab):
                    if m_ap is None:
                        nc.vector.tensor_copy(out=zslab[:, w, 0:fout],
                                              in_=zpre[:, w, 0:fout])
                        return
                    mt = wk.tile([P, fin], bf, tag="mt")
                    nc.scalar.activation(
                        out=mt[:], in_=m_ap, func=AF.Copy,
                        scale=neg2d2_own[:, w:w + 1])
                    psz2 = psz.tile([P, fout], f32, tag="psz", name="psz2")
                    for o in range(nch_in):
                        kp = min(P, fin - o * P)
                        pt = pstr.tile([P, P], bf, tag="pt")
                        nc.tensor.transpose(
                            out=pt[:kp, :], in_=mt[:, o * P:o * P + kp],
                            identity=identb_t[:])
                        mtt = wk.tile([P, P], bf, tag="mtt")
                        nc.scalar.activation(out=mtt[:kp, :], in_=pt[:kp, :],
                                             func=AF.Copy)
                        nc.tensor.matmul(
                            psz2[:], mtt[:kp, :], lw["wc"][o][0][:kp, :],
                            start=(o == 0), stop=(o == nch_in - 1))
                    nc.vector.tensor_tensor(
                        out=zslab[:, w, 0:fout], in0=psz2[:],
                        in1=zpre[:, w, 0:fout], op=OP.add)

                zc = zcon2 if p2_paired else zcon
                zc_v = zc[:, :].rearrange("(w p) f -> p w f", p=P)

                def z_half(half, fout=fout, zslab=zslab, zc_v=zc_v,
                           tzA=cfg["tzA"], tzB=cfg["tzB"], zc=zc,
                           p2_paired=p2_paired):
                    if half == 0:
                        nc.sync.dma_start(out=zc_v[:, 0:NBA, 0:fout],
                                          in_=zslab[:, 0:NBA, 0:fout])
                        nc.gpsimd.collective_compute(
                            "AllGather", OP.bypass,
                            replica_groups=[list(range(NCORES))],
                            ins=[zc[0:LOCA, :].opt()],
                            outs=[tzA[:].opt()])
                    else:
                        nc.sync.dma_start(out=zc_v[:, NBA:NB, 0:fout],
                                          in_=zslab[:, NBA:NB, 0:fout])
                        nc.gpsimd.collective_compute(
                            "AllGather", OP.bypass,
                            replica_groups=[list(range(NCORES))],
                            ins=[zc[LOCA:RB, :].opt()],
                            outs=[tzB[:].opt()])

                run_propagate(cfg["tinA"][:, :], cfg["tinB"][:, :],
                              gpa if p1_paired else gia,
                              gpb if p1_paired else gib,
                              cfg["tin_w"], fin,
                              CALL_SG_X if li == 0 else CALL_SG_T,
                              p1_paired, p1_win, z_half)

                # ---- P2 pre-pass: dpre[w] = (H Wa + b)[own w] ----
                dpre = slb.tile([P, NB, F1], bf, tag="pre", name="dpre")
                for w in range(NB):
                    psd_t = psz.tile([P, fout], f32, tag="psz",
                                     name="psd_t")
                    if li == 0:
                        for o, kp in XCH:
                            xw = wk.tile([P, P], bf, tag="xw")
                            nc.sync.dma_start(
                                out=xw[:kp, :],
                                in_=xTbf[o * P:o * P + kp,
                                         w * P:(w + 1) * P])
                            nc.tensor.matmul(
                                psd_t[:], xw[:kp, :],
                                lw["wa"][o][0][:kp, :],
                                start=(o == 0), stop=False)
                    else:
                        for o in range(nch_in):
                            ht = wk.tile([P, P], bf, tag="hoT")
                            nc.sync.dma_start(
                                out=ht[:],
                                in_=hown[w * P:(w + 1) * P, 0:P],
                                transpose=True)
                            kp = lw["wa"][o][1]
                            nc.tensor.matmul(
                                psd_t[:], ht[:kp, :],
                                lw["wa"][o][0][:kp, :],
                                start=(o == 0), stop=False)
                    nc.tensor.matmul(psd_t[:], ones1[:, :],
                                     b_t[li][:, :], start=False, stop=True)
                    nc.scalar.activation(out=dpre[:, w, 0:fout],
                                         in_=psd_t[:], func=AF.Copy)

                # ---- P2: M2 -> H' ----
                hslab = htslab = pooled = None
                if li < 2:
                    hslab = slb.tile([P, NB, F1], bf, tag="hslab",
                                     name="hslab")
                    htslab = slb.tile([P, NB, F1], bf, tag="htslab",
                                      name="htslab")
                else:
                    pooled = psfin.tile([P, F3 + 1], f32, tag="pooled",
                                        name="pooled")

                def p2_win(w, m_ap, li=li, fout=fout, dpre=dpre,
                           hslab=hslab, htslab=htslab, pooled=pooled):
                    if m_ap is not None:
                        v2 = wk.tile([P, fout], f32, tag="v2")
                        nc.scalar.activation(
                            out=v2[:], in_=m_ap, func=AF.Copy,
                            scale=negd_own[:, w:w + 1])
                        s = wk.tile([P, fout], f32, tag="s")
                        nc.vector.tensor_tensor(
                            out=s[:], in0=v2[:], in1=dpre[:, w, 0:fout],
                            op=OP.add)
                        src_ap = s[:]
                    else:
                        src_ap = dpre[:, w, 0:fout]
                    if li < 2:
                        nc.scalar.activation(out=hslab[:, w, 0:fout],
                                             in_=src_ap, func=AF.Relu)
                        nc.scalar.activation(
                            out=htslab[:, w, 0:fout], in_=src_ap,
                            func=AF.Relu, scale=dinv_own[:, w:w + 1])
                    else:
                        r33 = wk.tile([P, F3 + 1], f32, tag="r33")
                        nc.vector.memset(r33[:], 1.0)
                        nc.scalar.activation(out=r33[:, 0:F3], in_=src_ap,
                                             func=AF.Relu)
                        Bp = wk.tile([P, P], f32, tag="Bp")
                        nc.vector.tensor_scalar(
                            out=Bp[:], in0=iotaf_t[:],
                            scalar1=batchc_t[:, w:w + 1], scalar2=None,
                            op0=OP.is_equal)
                        nc.tensor.matmul(pooled[:], Bp[:], r33[:],
                                         start=(w == 0), stop=(w == NB - 1))

                hc_v = hcon[:, :].rearrange("(w p) f -> p w f", p=P)
                hc2_v = hcon2[:, :].rearrange("(w p) f -> p w f", p=P)
                ho_v = hown[:, :].rearrange("(w p) f -> p w f", p=P)

                def h_half(half, li=li, fout=fout, hslab=hslab,
                           htslab=htslab, thA=cfg["thA"], thB=cfg["thB"]):
                    if li >= 2:
                        return
                    lohiw = (0, NBA) if half == 0 else (NBA, NB)
                    w0, w1_ = lohiw
                    nc.sync.dma_start(out=ho_v[:, w0:w1_, 0:fout],
                                      in_=hslab[:, w0:w1_, 0:fout])
                    nc.sync.dma_start(out=hc_v[:, w0:w1_, 0:fout],
                                      in_=htslab[:, w0:w1_, 0:fout])
                    nxt_paired = li == 1
                    if nxt_paired:
                        nc.sync.dma_start(out=hc2_v[:, w0:w1_, 0:fout],
                                          in_=htslab[:, w0:w1_, 0:fout])
                        con = hcon2
                    else:
                        con = hcon
                    if half == 0:
                        nc.gpsimd.collective_compute(
                            "AllGather", OP.bypass,
                            replica_groups=[list(range(NCORES))],
                            ins=[con[0:LOCA, :].opt()],
                            outs=[thA[:].opt()])
                    else:
                        nc.gpsimd.collective_compute(
                            "AllGather", OP.bypass,
                            replica_groups=[list(range(NCORES))],
                            ins=[con[LOCA:RB, :].opt()],
                            outs=[thB[:].opt()])

                run_propagate(cfg["tzA"][:, :], cfg["tzB"][:, :],
                              gpa if p2_paired else gia,
                              gpb if p2_paired else gib,
                              TW, fout, CALL_SG_T, p2_paired, p2_win,
                              h_half if li < 2 else None)

            # ---------------- pooled mean + head ----------------
            psb = wk1.tile([P, F3 + 1], f32, tag="psb")
            nc.vector.tensor_copy(out=psb[:], in_=pooled[:])
            nc.sync.dma_start(out=arin[:, :], in_=psb[:])
            nc.gpsimd.collective_compute(
                "AllReduce", OP.add, replica_groups=[list(range(NCORES))],
                ins=[arin[:, :].opt()], outs=[arout[:, :].opt()])
            pr = wk1.tile([P, F3 + 1], f32, tag="pr")
            nc.sync.dma_start(out=pr[:], in_=arout[:, :])
            cmax = wk1.tile([P, 1], f32, tag="cmax")
            nc.vector.tensor_scalar_max(cmax[:], pr[:, F3:F3 + 1], 1.0)
            rcp = wk1.tile([P, 1], f32, tag="rcpf")
            nc.vector.reciprocal(rcp[:], cmax[:])
            pm = wk1.tile([P, F3], f32, tag="pm")
            nc.scalar.activation(out=pm[:], in_=pr[:, 0:F3], func=AF.Copy,
                                 scale=rcp[:, 0:1])
            ptp = pstr.tile([P, P], f32, tag="ptr")
            nc.tensor.transpose(out=ptp[:F3, :], in_=pm[:],
                                identity=identf_t[:])
            pmT = wk1.tile([P, P], f32, tag="pmT")
            nc.scalar.activation(out=pmT[:F3, :], in_=ptp[:F3, :],
                                 func=AF.Copy)
            psy = psfin.tile([P, 2], f32, tag="psy")
            nc.tensor.matmul(psy[:], pmT[:F3, :], wlt[:F3, :], start=True,
                             stop=True)
            yt = wk1.tile([P, 2], f32, tag="yt")
            nc.vector.tensor_tensor(out=yt[:], in0=psy[:], in1=blt[:],
                                    op=OP.add)
            nc.sync.dma_start(out=y[:, :], in_=yt[:])

    nc.compile()
    return nc


def kernel(**inputs):
    hp, in_maps = _prep(inputs)
    nc = _build(hp)
    from concourse import bass_utils
    res = bass_utils.run_bass_kernel_spmd(nc, in_maps,
                                          core_ids=list(range(NCORES)))
    return np.asarray(res.results[0]["y"], np.float32)


# revision 12
# speedup vs baseline: 1.0295x; 1.0295x over previous
"""GCN (3x ChebConv K=3 + global mean pool + linear head) on 8 Trainium2
NeuronCores via Bass/Tile — matmul-scatter design.

Per layer (fin -> fout, weights W[0..2]):
    out = H (W0 - W2) + L (H W1 + 2 L (H W2)),   L = -D^-1/2 A D^-1/2
Both L applications are gather + B-matrix matmul-scatter:
  - edges dst-partitioned across 8 cores, grouped per 128-row dst window
    into fixed-count 128-edge subgroups (max over cores -> SPMD-invariant),
  - gather src rows from a replicated bf16 table via gpsimd dma_gather
    (>=256B rows), scatter via PE matmul with a DVE-built selection matrix
    B[e, r] = ea_e * (dstloc_e == r) accumulating in PSUM per window,
  - L(H W2) = (L H) W2: the first L gathers the H table itself and applies
    W2 per window after the scatter, so no intermediate U table exists.
Narrow tables (width 64) are packed two-logical-rows-per-256B-row; the
gather uses idx g//2 and the scatter splits each subgroup into two
parity-masked B matmuls against the left/right half of the gathered pair.
Dense per-window matmuls are precomputed into bf16 slabs (zpre/dpre) so
they overlap the AllGathers. Tables are dinv-prescaled; dinv comes from a
host-packed ea slot layout reduced on DVE.
"""
import sys
sys.path.insert(0, "/opt/trn_rl_repo")
import numpy as np

P = 128
NCORES = 8
N, E, FIN, NG = 50000, 500000, 160, 128
RPC = N // NCORES            # 6250
NB = (RPC + P - 1) // P      # 49
RB = NB * P                  # 6272
NTOT = RB * NCORES           # 50176
F1, F2, F3 = 128, 64, 32
XW = 256                     # x~ table cols (bf16; 160 real)
TW = 128                     # wide table cols (bf16)
LO = 32768                   # int16 gather table split row
CALL_SG_X = 32               # subgroups per gather call, 512B rows
CALL_SG_T = 64               # subgroups per gather call, 256B rows


def _wrap16(flat):
    w = np.ascontiguousarray(flat.reshape(-1, 16).T).astype(np.int16)
    return np.tile(w, (8, 1))


def _prep(inputs):
    x = np.asarray(inputs["x"], np.float32)
    ei = np.asarray(inputs["edge_index"]).astype(np.int64)
    ea = np.asarray(inputs["edge_attr"], np.float32)
    batch = np.asarray(inputs["batch"]).astype(np.int64)
    Ws = [np.asarray(inputs[k], np.float32) for k in ("W1", "W2", "W3")]
    bs = [np.asarray(inputs[k], np.float32) for k in ("b1", "b2", "b3")]
    Wl = np.asarray(inputs["Wl"], np.float32)
    bl = np.asarray(inputs["bl"], np.float32)

    src, dst = ei[0], ei[1]
    g = (src // RPC) * RB + (src % RPC)      # table row of src
    owner = dst // RPC
    dstloc = dst % RPC
    wid = dstloc // P
    wloc = dstloc % P

    # --- per (core, window) lo/hi edge lists ---
    lists = [[None] * NB for _ in range(NCORES)]
    for c in range(NCORES):
        mc = np.nonzero(owner == c)[0]
        wsub = wid[mc]
        order = np.argsort(wsub, kind="stable")
        mc = mc[order]
        bounds = np.searchsorted(wsub[order], np.arange(NB + 1))
        for w in range(NB):
            m = mc[bounds[w]:bounds[w + 1]]
            m = m[np.argsort(g[m], kind="stable")]
            nlo = int(np.searchsorted(g[m], LO))
            lists[c][w] = (m[:nlo], m[nlo:])

    S_lo = [max(-(-len(lists[c][w][0]) // P) for c in range(NCORES))
            for w in range(NB)]
    S_hi = [max(-(-len(lists[c][w][1]) // P) for c in range(NCORES))
            for w in range(NB)]
    NSUBLO, NSUBHI = sum(S_lo), sum(S_hi)
    NSUB = NSUBLO + NSUBHI
    lo_pre = np.concatenate([[0], np.cumsum(S_lo)]).astype(int)
    hi_pre = np.concatenate([[0], np.cumsum(S_hi)]).astype(int)

    # schedule: per window, list of (stream, q) in consumption order
    sched = []
    for w in range(NB):
        subs = [("lo", lo_pre[w] + j) for j in range(S_lo[w])]
        subs += [("hi", hi_pre[w] + j) for j in range(S_hi[w])]
        sched.append(subs)

    # --- deg slot layout (own src rows) ---
    srcloc_all = src % RPC
    src_owner = src // RPC
    cnts = np.zeros((NCORES, RPC), np.int64)
    for c in range(NCORES):
        cnts[c] = np.bincount(srcloc_all[src_owner == c], minlength=RPC)
    K_DEG = int(cnts.max())

    # --- replicated x table (row-major, padded rows zero) ---
    xfull = np.zeros((NTOT, FIN), np.float32)
    for c in range(NCORES):
        xfull[c * RB:c * RB + RPC] = x[c * RPC:(c + 1) * RPC]

    iota_f = np.tile(np.arange(P, dtype=np.float32), (P, 1))
    ident_f = np.eye(P, dtype=np.float32)
    import ml_dtypes
    bf16 = ml_dtypes.bfloat16

    in_maps = []
    for c in range(NCORES):
        lo_idx = np.zeros(max(NSUBLO, 1) * P, np.int64)
        hi_idx = np.zeros(max(NSUBHI, 1) * P, np.int64)
        pair_idx = np.zeros(NSUB * P, np.int64)
        bcol = np.zeros((P, NSUB), np.float32)
        eacol = np.zeros((P, NSUB), np.float32)
        eacolA = np.zeros((P, NSUB), np.float32)
        eacolB = np.zeros((P, NSUB), np.float32)

        def fill(m, col):
            n = len(m)
            pair_idx[col * P:col * P + n] = g[m] // 2
            bcol[:n, col] = wloc[m]
            eacol[:n, col] = ea[m]
            par = (g[m] % 2).astype(np.float32)
            eacolA[:n, col] = ea[m] * (1.0 - par)
            eacolB[:n, col] = ea[m] * par

        for w in range(NB):
            elo, ehi = lists[c][w]
            for j in range(S_lo[w]):
                m = elo[j * P:(j + 1) * P]
                q = lo_pre[w] + j
                lo_idx[q * P:q * P + len(m)] = g[m]
                fill(m, q)
            for j in range(S_hi[w]):
                m = ehi[j * P:(j + 1) * P]
                q = hi_pre[w] + j
                hi_idx[q * P:q * P + len(m)] = g[m] - LO
                fill(m, NSUBLO + q)

        dslot = np.zeros((P, NB * K_DEG), np.float32)
        me = np.nonzero(src_owner == c)[0]
        slot_ctr = np.zeros(RPC, np.int64)
        locs = srcloc_all[me]
        for e, loc in zip(me, locs):
            s = slot_ctr[loc]
            slot_ctr[loc] += 1
            dslot[loc % P, (loc // P) * K_DEG + s] = ea[e]

        batchc = np.full((P, NB), 999.0, np.float32)
        blk = np.full(RB, 999.0, np.float32)
        blk[:RPC] = batch[c * RPC:(c + 1) * RPC]
        batchc[:, :] = blk.reshape(NB, P).T

        xT_own = np.zeros((FIN, RB), np.float32)
        xT_own[:, :RPC] = x[c * RPC:(c + 1) * RPC].T

        im = dict(
            xfull=xfull, xT=xT_own,
            glo=_wrap16(lo_idx), ghi=_wrap16(hi_idx),
            gpair=_wrap16(pair_idx),
            bcol=bcol, eacol=eacol, eacolA=eacolA, eacolB=eacolB,
            dslot=dslot, batchc=batchc,
            w1=Ws[0], w2=Ws[1], w3=Ws[2], wl=Wl,
            b1=bs[0][None, :], b2=bs[1][None, :], b3=bs[2][None, :],
            blb=np.tile(bl, (P, 1)),
            iotaf=iota_f, iotab=iota_f.astype(bf16),
            identb=ident_f.astype(bf16), identf=ident_f,
        )
        in_maps.append(im)

    hp = dict(NSUBLO=NSUBLO, NSUBHI=NSUBHI, NSUB=NSUB, K_DEG=K_DEG,
              S_lo=S_lo, S_hi=S_hi, sched=sched)
    return hp, in_maps


def _build(hp):
    import concourse.bacc as bacc
    import concourse.tile as tile
    import concourse.mybir as mybir
    from concourse import library_config
    dt = mybir.dt
    AF = mybir.ActivationFunctionType
    OP = mybir.AluOpType
    f32, bf = dt.float32, dt.bfloat16

    NSUBLO, NSUBHI, NSUB = hp["NSUBLO"], hp["NSUBHI"], hp["NSUB"]
    K_DEG, sched = hp["K_DEG"], hp["sched"]

    nc = bacc.Bacc("TRN2", target_bir_lowering=False, debug=False,
                   num_devices=NCORES, dynamic_dma_scratch_size=24576)

    xfull = nc.dram_tensor("xfull", [NTOT, FIN], f32, kind="ExternalInput")
    xT = nc.dram_tensor("xT", [FIN, RB], f32, kind="ExternalInput")
    glo = nc.dram_tensor("glo", [P, max(NSUBLO, 1) * 8], dt.int16,
                         kind="ExternalInput")
    ghi = nc.dram_tensor("ghi", [P, max(NSUBHI, 1) * 8], dt.int16,
                         kind="ExternalInput")
    gpair = nc.dram_tensor("gpair", [P, NSUB * 8], dt.int16,
                           kind="ExternalInput")
    bcol = nc.dram_tensor("bcol", [P, NSUB], f32, kind="ExternalInput")
    eacol = nc.dram_tensor("eacol", [P, NSUB], f32, kind="ExternalInput")
    eacolA = nc.dram_tensor("eacolA", [P, NSUB], f32, kind="ExternalInput")
    eacolB = nc.dram_tensor("eacolB", [P, NSUB], f32, kind="ExternalInput")
    dslot = nc.dram_tensor("dslot", [P, NB * K_DEG], f32,
                           kind="ExternalInput")
    batchc = nc.dram_tensor("batchc", [P, NB], f32, kind="ExternalInput")
    w1 = nc.dram_tensor("w1", [3, FIN, F1], f32, kind="ExternalInput")
    w2 = nc.dram_tensor("w2", [3, F1, F2], f32, kind="ExternalInput")
    w3 = nc.dram_tensor("w3", [3, F2, F3], f32, kind="ExternalInput")
    wl = nc.dram_tensor("wl", [F3, 2], f32, kind="ExternalInput")
    b1 = nc.dram_tensor("b1", [1, F1], f32, kind="ExternalInput")
    b2 = nc.dram_tensor("b2", [1, F2], f32, kind="ExternalInput")
    b3 = nc.dram_tensor("b3", [1, F3], f32, kind="ExternalInput")
    blb = nc.dram_tensor("blb", [P, 2], f32, kind="ExternalInput")
    iotaf = nc.dram_tensor("iotaf", [P, P], f32, kind="ExternalInput")
    iotab = nc.dram_tensor("iotab", [P, P], bf, kind="ExternalInput")
    identb = nc.dram_tensor("identb", [P, P], bf, kind="ExternalInput")
    identf = nc.dram_tensor("identf", [P, P], f32, kind="ExternalInput")
    y = nc.dram_tensor("y", [P, 2], f32, kind="ExternalOutput")

    with tile.TileContext(nc) as tc:
        with tc.tile_pool(name="cst", bufs=1) as cst, \
             tc.tile_pool(name="wk", bufs=3) as wk, \
             tc.tile_pool(name="wk1", bufs=1) as wk1, \
             tc.tile_pool(name="bp", bufs=3) as bp, \
             tc.tile_pool(name="slb", bufs=1) as slb, \
             tc.tile_pool(name="vlo", bufs=2) as vlo, \
             tc.tile_pool(name="vhi", bufs=2) as vhi, \
             tc.tile_pool(name="psm", bufs=2, space="PSUM") as psm, \
             tc.tile_pool(name="psz", bufs=2, space="PSUM") as psz, \
             tc.tile_pool(name="pstr", bufs=1, space="PSUM") as pstr, \
             tc.tile_pool(name="psfin", bufs=1, space="PSUM") as psfin, \
             tc.tile_pool(name="dram", bufs=1, space="DRAM") as dram:

            nc.gpsimd.load_library(library_config.mlp)

            # ---------------- dram tables ----------------
            Tx = dram.tile([NTOT, XW], bf, tag="Tx", name="Tx")
            Tz1 = dram.tile([NTOT, TW], bf, tag="Tz1", name="Tz1",
                            addr_space="Shared")
            Th1 = dram.tile([NTOT, TW], bf, tag="Th1", name="Th1",
                            addr_space="Shared")
            Tz2 = dram.tile([NTOT // 2, TW], bf, tag="Tz2", name="Tz2",
                            addr_space="Shared")
            Th2 = dram.tile([NTOT // 2, TW], bf, tag="Th2", name="Th2",
                            addr_space="Shared")
            Tz3 = dram.tile([NTOT // 2, TW], bf, tag="Tz3", name="Tz3",
                            addr_space="Shared")
            zcon = dram.tile([RB, TW], bf, tag="zcon", name="zcon")
            zcon2 = dram.tile([RB, TW // 2], bf, tag="zcon2", name="zcon2")
            hcon = dram.tile([RB, TW], bf, tag="hcon", name="hcon")
            hcon2 = dram.tile([RB, TW // 2], bf, tag="hcon2", name="hcon2")
            hown = dram.tile([RB, TW], bf, tag="hown", name="hown")
            xTbf = dram.tile([FIN, RB], bf, tag="xTbf", name="xTbf")
            degsh = dram.tile([RB, 1], f32, tag="degsh", name="degsh")
            degf = dram.tile([NTOT, 1], f32, tag="degf", name="degf")
            arin = dram.tile([P, F3 + 1], f32, tag="arin", name="arin")
            arout = dram.tile([P, F3 + 1], f32, tag="arout", name="arout")

            # ---------------- consts ----------------
            iotab_t = cst.tile([P, P], bf)
            nc.sync.dma_start(out=iotab_t[:], in_=iotab[:, :])
            iotaf_t = cst.tile([P, P], f32)
            nc.sync.dma_start(out=iotaf_t[:], in_=iotaf[:, :])
            identb_t = cst.tile([P, P], bf)
            nc.sync.dma_start(out=identb_t[:], in_=identb[:, :])
            identf_t = cst.tile([P, P], f32)
            nc.sync.dma_start(out=identf_t[:], in_=identf[:, :])
            bcol_t = cst.tile([P, NSUB], f32)
            nc.sync.dma_start(out=bcol_t[:], in_=bcol[:, :])
            eacol_t = cst.tile([P, NSUB], f32)
            nc.sync.dma_start(out=eacol_t[:], in_=eacol[:, :])
            eaA_t = cst.tile([P, NSUB], f32)
            nc.sync.dma_start(out=eaA_t[:], in_=eacolA[:, :])
            eaB_t = cst.tile([P, NSUB], f32)
            nc.sync.dma_start(out=eaB_t[:], in_=eacolB[:, :])
            glo_t = cst.tile([P, max(NSUBLO, 1) * 8], dt.int16)
            nc.sync.dma_start(out=glo_t[:], in_=glo[:, :])
            ghi_t = cst.tile([P, max(NSUBHI, 1) * 8], dt.int16)
            nc.sync.dma_start(out=ghi_t[:], in_=ghi[:, :])
            gpair_t = cst.tile([P, NSUB * 8], dt.int16)
            nc.sync.dma_start(out=gpair_t[:], in_=gpair[:, :])
            batchc_t = cst.tile([P, NB], f32)
            nc.sync.dma_start(out=batchc_t[:], in_=batchc[:, :])
            ones1 = cst.tile([1, P], f32)
            nc.vector.memset(ones1[:], 1.0)
            blt = cst.tile([P, 2], f32)
            nc.sync.dma_start(out=blt[:], in_=blb[:, :])
            wlt = cst.tile([P, 2], f32)
            nc.sync.dma_start(out=wlt[:F3, :], in_=wl[:, :])
            b_t = []
            for bb, fo in ((b1, F1), (b2, F2), (b3, F3)):
                t = cst.tile([1, fo], f32, tag=f"b{fo}")
                nc.sync.dma_start(out=t[:], in_=bb[:, :])
                b_t.append(t)

            # weights -> bf16 chunk tiles: Wa = W0 - W2, Wb = W1, Wc = W2
            layer_w = []
            for li, (wt_, fin, fo) in enumerate(
                    ((w1, FIN, F1), (w2, F1, F2), (w3, F2, F3))):
                nch = (fin + P - 1) // P
                was, wbs, wcs = [], [], []
                for o in range(nch):
                    kp = min(P, fin - o * P)
                    t0 = wk.tile([P, fo], f32, tag="wld", bufs=2)
                    nc.sync.dma_start(out=t0[:kp, :],
                                      in_=wt_[0, o * P:o * P + kp, :])
                    t2 = wk.tile([P, fo], f32, tag="wld", bufs=2)
                    nc.sync.dma_start(out=t2[:kp, :],
                                      in_=wt_[2, o * P:o * P + kp, :])
                    t1 = wk.tile([P, fo], f32, tag="wld", bufs=2)
                    nc.sync.dma_start(out=t1[:kp, :],
                                      in_=wt_[1, o * P:o * P + kp, :])
                    wa = cst.tile([P, fo], bf, tag=f"wa{li}_{o}")
                    nc.vector.tensor_tensor(out=wa[:kp, :], in0=t0[:kp, :],
                                            in1=t2[:kp, :], op=OP.subtract)
                    wb_ = cst.tile([P, fo], bf, tag=f"wb{li}_{o}")
                    nc.vector.tensor_copy(out=wb_[:kp, :], in_=t1[:kp, :])
                    wc_ = cst.tile([P, fo], bf, tag=f"wc{li}_{o}")
                    nc.vector.tensor_copy(out=wc_[:kp, :], in_=t2[:kp, :])
                    was.append((wa, kp))
                    wbs.append((wb_, kp))
                    wcs.append((wc_, kp))
                layer_w.append(dict(wa=was, wb=wbs, wc=wcs))

            # xT -> bf16 DRAM copy (lhsT source for l=0 dense mms)
            XCH = [(0, P), (1, FIN - P)]
            for o, st_tag in ((0, "hslab"), (1, "htslab")):
                kp = min(P, FIN - o * P)
                xstage = slb.tile([P, RB], bf, tag=st_tag,
                                  name=f"xstage{o}")
                nc.gpsimd.dma_start(out=xstage[:kp, :],
                                    in_=xT[o * P:o * P + kp, :])
                nc.sync.dma_start(out=xTbf[o * P:o * P + kp, :],
                                  in_=xstage[:kp, :])

            # ---------------- deg / dinv ----------------
            degsb = wk1.tile([P, NB], f32, tag="degsb")
            CH_D = 7
            for c0 in range(0, NB, CH_D):
                ch = min(CH_D, NB - c0)
                t = wk.tile([P, CH_D * K_DEG], f32, tag="dgl", bufs=2)
                nc.sync.dma_start(
                    out=t[:, :ch * K_DEG],
                    in_=dslot[:, c0 * K_DEG:(c0 + ch) * K_DEG])
                nc.vector.tensor_reduce(
                    out=degsb[:, c0:c0 + ch, None],
                    in_=t[:, :ch * K_DEG].rearrange("p (b k) -> p b k",
                                                    k=K_DEG),
                    axis=mybir.AxisListType.X, op=OP.add)
            nc.sync.dma_start(
                out=degsh[:].rearrange("(b p) c -> p (b c)", p=P),
                in_=degsb[:])
            nc.gpsimd.collective_compute(
                "AllGather", OP.bypass, replica_groups=[list(range(NCORES))],
                ins=[degsh[:, :].opt()], outs=[degf[:, :].opt()])

            def dinv_of(deg_ap, cols, tag):
                m = wk1.tile([P, cols], f32, tag=tag + "m")
                nc.vector.tensor_scalar(out=m[:], in0=deg_ap, scalar1=0.0,
                                        scalar2=None, op0=OP.is_le)
                safe = wk1.tile([P, cols], f32, tag=tag + "s")
                nc.vector.tensor_tensor(out=safe[:], in0=deg_ap, in1=m[:],
                                        op=OP.add)
                sq = wk1.tile([P, cols], f32, tag=tag + "q")
                nc.scalar.activation(out=sq[:], in_=safe[:], func=AF.Sqrt)
                rcp = wk1.tile([P, cols], f32, tag=tag + "r")
                nc.vector.reciprocal(rcp[:], sq[:])
                gm = wk1.tile([P, cols], f32, tag=tag + "g")
                nc.vector.tensor_scalar(out=gm[:], in0=deg_ap, scalar1=0.0,
                                        scalar2=None, op0=OP.is_gt)
                dv = cst.tile([P, cols], f32, tag=tag + "d")
                nc.vector.tensor_tensor(out=dv[:], in0=rcp[:], in1=gm[:],
                                        op=OP.mult)
                return dv

            dinv_own = dinv_of(degsb[:], NB, "dow")
            negd_own = cst.tile([P, NB], f32)
            nc.vector.tensor_scalar_mul(negd_own[:], dinv_own[:], -1.0)
            d2 = wk1.tile([P, NB], f32, tag="d2")
            nc.vector.tensor_tensor(out=d2[:], in0=dinv_own[:],
                                    in1=dinv_own[:], op=OP.mult)
            neg2d2_own = cst.tile([P, NB], f32)
            nc.vector.tensor_scalar_mul(neg2d2_own[:], d2[:], -2.0)

            NCOLT = NTOT // P
            degfsb = wk1.tile([P, NCOLT], f32, tag="degfsb")
            nc.sync.dma_start(
                out=degfsb[:],
                in_=degf[:, 0:1].rearrange("(b p) c -> p (b c)", p=P))
            dinv_full = dinv_of(degfsb[:], NCOLT, "dfu")

            # ---------------- x~ table build ----------------
            xf_v = xfull[:, :].rearrange("(t p) f -> p t f", p=P)
            tx_v = Tx[:].rearrange("(t p) f -> p t f", p=P)
            CH_X = 14
            for t0 in range(0, NCOLT, CH_X):
                ch = min(CH_X, NCOLT - t0)
                xt_ = wk.tile([P, CH_X, FIN], f32, tag="xld", bufs=2)
                nc.sync.dma_start(out=xt_[:, :ch, :],
                                  in_=xf_v[:, t0:t0 + ch, :])
                xs = wk.tile([P, CH_X, FIN], bf, tag="xsc", bufs=2)
                for t in range(ch):
                    nc.scalar.activation(
                        out=xs[:, t, :], in_=xt_[:, t, :], func=AF.Copy,
                        scale=dinv_full[:, t0 + t:t0 + t + 1])
                nc.sync.dma_start(out=tx_v[:, t0:t0 + ch, 0:FIN],
                                  in_=xs[:, :ch, :])

            # ---------------- propagate machinery ----------------
            def run_propagate(streams, twidth, realw, call_sg, win_fn,
                              paired):
                """streams: {name: (idx_tile, sub_offset, nsub, table_ap,
                pool)}; gathers + B-matmul scatter over all windows."""
                state = {}
                for stname, (gt, goff, nsub_s, tap, pool) in streams.items():
                    ncalls = -(-nsub_s // call_sg) if nsub_s else 0
                    state[stname] = dict(gt=gt, goff=goff, nsub=nsub_s,
                                         tap=tap, pool=pool, issued=0,
                                         tiles=[], ncalls=ncalls)

                def issue(st):
                    s = state[st]
                    a = s["issued"] * call_sg
                    b = min(a + call_sg, s["nsub"])
                    nsg = b - a
                    vt = s["pool"].tile([P, call_sg, twidth], bf,
                                        tag=f"v{st}", name=f"v{st}")
                    nc.gpsimd.dma_gather(
                        out_ap=vt[:, :nsg, :], in_ap=s["tap"],
                        idxs_ap=s["gt"][:, (s["goff"] + a) * 8:
                                        (s["goff"] + b) * 8],
                        num_idxs=nsg * P, num_idxs_reg=nsg * P,
                        elem_size=twidth, single_packet=False)
                    s["tiles"].append((vt, a))
                    s["issued"] += 1

                def get(st, q):
                    s = state[st]
                    while s["issued"] * call_sg <= q:
                        issue(st)
                    if s["issued"] < s["ncalls"] and \
                            q >= (s["issued"] - 1) * call_sg + call_sg // 2:
                        issue(st)
                    ci = q // call_sg
                    vt, a = s["tiles"][ci]
                    return vt, q - a

                for w in range(NB):
                    subs = sched[w]
                    psm_t = None
                    if subs:
                        psm_t = psm.tile([P, realw], f32, tag="psm",
                                         name="psm_t")
                        nmm = len(subs) * (2 if paired else 1)
                        i = 0
                        for st, q in subs:
                            col = q if st == "lo" else NSUBLO + q
                            vt, slot = get(st, q)
                            if paired:
                                for eat, off in ((eaA_t, 0), (eaB_t, 64)):
                                    B = bp.tile([P, P], bf, tag="B",
                                                name="B")
                                    nc.vector.tensor_scalar(
                                        out=B[:], in0=iotab_t[:],
                                        scalar1=bcol_t[:, col:col + 1],
                                        scalar2=eat[:, col:col + 1],
                                        op0=OP.is_equal, op1=OP.mult)
                                    nc.tensor.matmul(
                                        psm_t[:], B[:],
                                        vt[:, slot, off:off + realw],
                                        start=(i == 0), stop=(i == nmm - 1))
                                    i += 1
                            else:
                                B = bp.tile([P, P], bf, tag="B", name="B")
                                nc.vector.tensor_scalar(
                                    out=B[:], in0=iotab_t[:],
                                    scalar1=bcol_t[:, col:col + 1],
                                    scalar2=eacol_t[:, col:col + 1],
                                    op0=OP.is_equal, op1=OP.mult)
                                nc.tensor.matmul(
                                    psm_t[:], B[:], vt[:, slot, 0:realw],
                                    start=(i == 0), stop=(i == nmm - 1))
                                i += 1
                    win_fn(w, psm_t)

            def unpaired_streams(tbl):
                return {"lo": (glo_t, 0, NSUBLO, tbl[0:LO, :], vlo),
                        "hi": (ghi_t, 0, NSUBHI, tbl[LO:NTOT, :], vhi)}

            def paired_streams(tbl):
                return {"lo": (gpair_t, 0, NSUBLO, tbl[:, :], vlo),
                        "hi": (gpair_t, NSUBLO, NSUBHI, tbl[:, :], vhi)}

            # ---------------- layers ----------------
            layer_cfg = [
                dict(fin=FIN, fout=F1, p1_paired=False, p2_paired=False,
                     tin=Tx, tin_w=XW, tz=Tz1, th=Th1),
                dict(fin=F1, fout=F2, p1_paired=False, p2_paired=True,
                     tin=Th1, tin_w=TW, tz=Tz2, th=Th2),
                dict(fin=F2, fout=F3, p1_paired=True, p2_paired=True,
                     tin=Th2, tin_w=TW, tz=Tz3, th=None),
            ]

            for li, cfg in enumerate(layer_cfg):
                fin, fout = cfg["fin"], cfg["fout"]
                lw = layer_w[li]
                nch_in = (fin + P - 1) // P

                # ---- P1 pre-pass: zpre[w] = (H~ W1)[own w] ----
                zpre = slb.tile([P, NB, F1], bf, tag="pre", name="zpre")
                for w in range(NB):
                    psz1 = psz.tile([P, fout], f32, tag="psz", name="psz1")
                    if li == 0:
                        for o, kp in XCH:
                            xw = wk.tile([P, P], bf, tag="xw")
                            nc.sync.dma_start(
                                out=xw[:kp, :],
                                in_=xTbf[o * P:o * P + kp,
                                         w * P:(w + 1) * P])
                            nc.tensor.matmul(
                                psz1[:], xw[:kp, :], lw["wb"][o][0][:kp, :],
                                start=(o == 0), stop=(o == len(XCH) - 1))
                    else:
                        for o in range(nch_in):
                            ht = wk.tile([P, P], bf, tag="htT")
                            nc.sync.dma_start(
                                out=ht[:],
                                in_=hcon[w * P:(w + 1) * P,
                                         o * P:(o + 1) * P],
                                transpose=True)
                            kp = lw["wb"][o][1]
                            nc.tensor.matmul(
                                psz1[:], ht[:kp, :], lw["wb"][o][0][:kp, :],
                                start=(o == 0), stop=(o == nch_in - 1))
                    if li == 0:
                        nc.scalar.activation(
                            out=zpre[:, w, 0:fout], in_=psz1[:],
                            func=AF.Copy, scale=dinv_own[:, w:w + 1])
                    else:
                        nc.scalar.activation(out=zpre[:, w, 0:fout],
                                             in_=psz1[:], func=AF.Copy)

                # ---- P1: M1 -> Z~ own ----
                zslab = slb.tile([P, NB, F1], bf, tag="zslab", name="zslab")

                def p1_win(w, psm_t, lw=lw, fin=fin, fout=fout,
                           nch_in=nch_in, zpre=zpre, zslab=zslab):
                    if psm_t is None:
                        nc.vector.tensor_copy(out=zslab[:, w, 0:fout],
                                              in_=zpre[:, w, 0:fout])
                        return
                    mt = wk.tile([P, fin], bf, tag="mt")
                    nc.scalar.activation(
                        out=mt[:], in_=psm_t[:, 0:fin], func=AF.Copy,
                        scale=neg2d2_own[:, w:w + 1])
                    psz2 = psz.tile([P, fout], f32, tag="psz", name="psz2")
                    for o in range(nch_in):
                        kp = min(P, fin - o * P)
                        pt = pstr.tile([P, P], bf, tag="pt")
                        nc.tensor.transpose(
                            out=pt[:kp, :], in_=mt[:, o * P:o * P + kp],
                            identity=identb_t[:])
                        mtt = wk.tile([P, P], bf, tag="mtt")
                        nc.scalar.activation(out=mtt[:kp, :], in_=pt[:kp, :],
                                             func=AF.Copy)
                        nc.tensor.matmul(
                            psz2[:], mtt[:kp, :], lw["wc"][o][0][:kp, :],
                            start=(o == 0), stop=(o == nch_in - 1))
                    nc.vector.tensor_tensor(
                        out=zslab[:, w, 0:fout], in0=psz2[:],
                        in1=zpre[:, w, 0:fout], op=OP.add)

                run_propagate(
                    paired_streams(cfg["tin"]) if cfg["p1_paired"]
                    else unpaired_streams(cfg["tin"]),
                    cfg["tin_w"], fin,
                    CALL_SG_X if li == 0 else CALL_SG_T, p1_win,
                    cfg["p1_paired"])

                if cfg["p2_paired"]:
                    # packed contribution: [RB, 64] (real fout cols)
                    nc.sync.dma_start(
                        out=zcon2[:, :].rearrange("(w p) f -> p w f", p=P)
                        [:, :, 0:fout],
                        in_=zslab[:, :, 0:fout])
                    nc.gpsimd.collective_compute(
                        "AllGather", OP.bypass,
                        replica_groups=[list(range(NCORES))],
                        ins=[zcon2[:, :].opt()], outs=[cfg["tz"][:].opt()])
                else:
                    nc.sync.dma_start(
                        out=zcon[:, :].rearrange("(w p) f -> p w f", p=P)
                        [:, :, 0:fout],
                        in_=zslab[:, :, 0:fout])
                    nc.gpsimd.collective_compute(
                        "AllGather", OP.bypass,
                        replica_groups=[list(range(NCORES))],
                        ins=[zcon[:, :].opt()], outs=[cfg["tz"][:].opt()])

                # ---- P2 pre-pass: dpre[w] = (H Wa + b)[own w] ----
                dpre = slb.tile([P, NB, F1], bf, tag="pre", name="dpre")
                for w in range(NB):
                    psd_t = psz.tile([P, fout], f32, tag="psz", name="psd_t")
                    if li == 0:
                        for o, kp in XCH:
                            xw = wk.tile([P, P], bf, tag="xw")
                            nc.sync.dma_start(
                                out=xw[:kp, :],
                                in_=xTbf[o * P:o * P + kp,
                                         w * P:(w + 1) * P])
                            nc.tensor.matmul(
                                psd_t[:], xw[:kp, :], lw["wa"][o][0][:kp, :],
                                start=(o == 0), stop=False)
                    else:
                        for o in range(nch_in):
                            ht = wk.tile([P, P], bf, tag="hoT")
                            nc.sync.dma_start(
                                out=ht[:],
                                in_=hown[w * P:(w + 1) * P, 0:P],
                                transpose=True)
                            kp = lw["wa"][o][1]
                            nc.tensor.matmul(
                                psd_t[:], ht[:kp, :], lw["wa"][o][0][:kp, :],
                                start=(o == 0), stop=False)
                    nc.tensor.matmul(psd_t[:], ones1[:, :],
                                     b_t[li][:, :], start=False, stop=True)
                    nc.scalar.activation(out=dpre[:, w, 0:fout],
                                         in_=psd_t[:], func=AF.Copy)

                # ---- P2: M2 -> H' ----
                hslab = htslab = pooled = None
                if li < 2:
                    hslab = slb.tile([P, NB, F1], bf, tag="hslab",
                                     name="hslab")
                    htslab = slb.tile([P, NB, F1], bf, tag="htslab",
                                      name="htslab")
                else:
                    pooled = psfin.tile([P, F3 + 1], f32, tag="pooled",
                                        name="pooled")

                def p2_win(w, psm_t, li=li, fout=fout, dpre=dpre,
                           hslab=hslab, htslab=htslab, pooled=pooled):
                    if psm_t is not None:
                        v2 = wk.tile([P, fout], f32, tag="v2")
                        nc.scalar.activation(
                            out=v2[:], in_=psm_t[:, 0:fout], func=AF.Copy,
                            scale=negd_own[:, w:w + 1])
                        s = wk.tile([P, fout], f32, tag="s")
                        nc.vector.tensor_tensor(
                            out=s[:], in0=v2[:], in1=dpre[:, w, 0:fout],
                            op=OP.add)
                        src_ap = s[:]
                    else:
                        src_ap = dpre[:, w, 0:fout]
                    if li < 2:
                        nc.scalar.activation(out=hslab[:, w, 0:fout],
                                             in_=src_ap, func=AF.Relu)
                        nc.scalar.activation(
                            out=htslab[:, w, 0:fout], in_=src_ap,
                            func=AF.Relu, scale=dinv_own[:, w:w + 1])
                    else:
                        r33 = wk.tile([P, F3 + 1], f32, tag="r33")
                        nc.vector.memset(r33[:], 1.0)
                        nc.scalar.activation(out=r33[:, 0:F3], in_=src_ap,
                                             func=AF.Relu)
                        Bp = wk.tile([P, P], f32, tag="Bp")
                        nc.vector.tensor_scalar(
                            out=Bp[:], in0=iotaf_t[:],
                            scalar1=batchc_t[:, w:w + 1], scalar2=None,
                            op0=OP.is_equal)
                        nc.tensor.matmul(pooled[:], Bp[:], r33[:],
                                         start=(w == 0), stop=(w == NB - 1))

                run_propagate(
                    paired_streams(cfg["tz"]) if cfg["p2_paired"]
                    else unpaired_streams(cfg["tz"]),
                    TW, fout, CALL_SG_T, p2_win, cfg["p2_paired"])

                if li < 2:
                    nc.sync.dma_start(
                        out=hown[:, :].rearrange("(w p) f -> p w f", p=P)
                        [:, :, 0:fout],
                        in_=hslab[:, :, 0:fout])
                    if li == 0:
                        nc.sync.dma_start(
                            out=hcon[:, :].rearrange("(w p) f -> p w f",
                                                     p=P)[:, :, 0:fout],
                            in_=htslab[:, :, 0:fout])
                        nc.gpsimd.collective_compute(
                            "AllGather", OP.bypass,
                            replica_groups=[list(range(NCORES))],
                            ins=[hcon[:, :].opt()],
                            outs=[cfg["th"][:].opt()])
                    else:
                        # local full-width staging for next P1's transposes
                        nc.sync.dma_start(
                            out=hcon[:, :].rearrange("(w p) f -> p w f",
                                                     p=P)[:, :, 0:fout],
                            in_=htslab[:, :, 0:fout])
                        nc.sync.dma_start(
                            out=hcon2[:, :].rearrange("(w p) f -> p w f",
                                                      p=P)[:, :, 0:fout],
                            in_=htslab[:, :, 0:fout])
                        nc.gpsimd.collective_compute(
                            "AllGather", OP.bypass,
                            replica_groups=[list(range(NCORES))],
                            ins=[hcon2[:, :].opt()],
                            outs=[cfg["th"][:].opt()])

            # ---------------- pooled mean + head ----------------
            psb = wk1.tile([P, F3 + 1], f32, tag="psb")
            nc.vector.tensor_copy(out=psb[:], in_=pooled[:])
            nc.sync.dma_start(out=arin[:, :], in_=psb[:])
            nc.gpsimd.collective_compute(
                "AllReduce", OP.add, replica_groups=[list(range(NCORES))],
                ins=[arin[:, :].opt()], outs=[arout[:, :].opt()])
            pr = wk1.tile([P, F3 + 1], f32, tag="pr")
            nc.sync.dma_start(out=pr[:], in_=arout[:, :])
            cmax = wk1.tile([P, 1], f32, tag="cmax")
            nc.vector.tensor_scalar_max(cmax[:], pr[:, F3:F3 + 1], 1.0)
            rcp = wk1.tile([P, 1], f32, tag="rcpf")
            nc.vector.reciprocal(rcp[:], cmax[:])
            pm = wk1.tile([P, F3], f32, tag="pm")
            nc.scalar.activation(out=pm[:], in_=pr[:, 0:F3], func=AF.Copy,
                                 scale=rcp[:, 0:1])
            ptp = pstr.tile([P, P], f32, tag="ptr")
            nc.tensor.transpose(out=ptp[:F3, :], in_=pm[:],
                                identity=identf_t[:])
            pmT = wk1.tile([P, P], f32, tag="pmT")
            nc.scalar.activation(out=pmT[:F3, :], in_=ptp[:F3, :],
                                 func=AF.Copy)
            psy = psfin.tile([P, 2], f32, tag="psy")
            nc.tensor.matmul(psy[:], pmT[:F3, :], wlt[:F3, :], start=True,
                             stop=True)
            yt = wk1.tile([P, 2], f32, tag="yt")
            nc.vector.tensor_tensor(out=yt[:], in0=psy[:], in1=blt[:],
                                    op=OP.add)
            nc.sync.dma_start(out=y[:, :], in_=yt[:])

    nc.compile()
    return nc


def kernel(**inputs):
    hp, in_maps = _prep(inputs)
    nc = _build(hp)
    from concourse import bass_utils
    res = bass_utils.run_bass_kernel_spmd(nc, in_maps,
                                          core_ids=list(range(NCORES)))
    return np.asarray(res.results[0]["y"], np.float32)


# revision 13
# speedup vs baseline: 1.0719x; 1.0411x over previous
"""GCN (3x ChebConv K=3 + global mean pool + linear head) on 8 Trainium2
NeuronCores via Bass/Tile — matmul-scatter design.

Per layer (fin -> fout, weights W[0..2]):
    out = H (W0 - W2) + L (H W1 + 2 L (H W2)),   L = -D^-1/2 A D^-1/2
Both L applications are gather + B-matrix matmul-scatter:
  - edges dst-partitioned across 8 cores, grouped per 128-row dst window
    into fixed-count 128-edge subgroups (max over cores -> SPMD-invariant),
  - gather src rows from a replicated bf16 table via gpsimd dma_gather
    (>=256B rows), scatter via PE matmul with a DVE-built selection matrix
    B[e, r] = ea_e * (dstloc_e == r) accumulating in PSUM per window,
  - L(H W2) = (L H) W2: the first L gathers the H table itself and applies
    W2 per window after the scatter, so no intermediate U table exists.
Narrow tables (width 64) are packed two-logical-rows-per-256B-row; the
gather uses idx g//2 and the scatter splits each subgroup into two
parity-masked B matmuls against the left/right half of the gathered pair.
Dense per-window matmuls are precomputed into bf16 slabs (zpre/dpre) so
they overlap the AllGathers. Tables are dinv-prescaled; dinv comes from a
host-packed ea slot layout reduced on DVE.
"""
import sys
sys.path.insert(0, "/opt/trn_rl_repo")
import numpy as np

P = 128
NCORES = 8
N, E, FIN, NG = 50000, 500000, 160, 128
RPC = N // NCORES            # 6250
NB = (RPC + P - 1) // P      # 49
RB = NB * P                  # 6272
NTOT = RB * NCORES           # 50176
F1, F2, F3 = 128, 64, 32
XW = 256                     # x~ table cols (bf16; 160 real)
TW = 128                     # wide table cols (bf16)
LO = 32768                   # int16 gather table split row
CALL_SG_X = 32               # subgroups per gather call, 512B rows
CALL_SG_T = 64               # subgroups per gather call, 256B rows


def _wrap16(flat):
    w = np.ascontiguousarray(flat.reshape(-1, 16).T).astype(np.int16)
    return np.tile(w, (8, 1))


def _prep(inputs):
    x = np.asarray(inputs["x"], np.float32)
    ei = np.asarray(inputs["edge_index"]).astype(np.int64)
    ea = np.asarray(inputs["edge_attr"], np.float32)
    batch = np.asarray(inputs["batch"]).astype(np.int64)
    Ws = [np.asarray(inputs[k], np.float32) for k in ("W1", "W2", "W3")]
    bs = [np.asarray(inputs[k], np.float32) for k in ("b1", "b2", "b3")]
    Wl = np.asarray(inputs["Wl"], np.float32)
    bl = np.asarray(inputs["bl"], np.float32)

    src, dst = ei[0], ei[1]
    g = (src // RPC) * RB + (src % RPC)      # table row of src
    owner = dst // RPC
    dstloc = dst % RPC
    wid = dstloc // P
    wloc = dstloc % P

    # --- per (core, window) lo/hi edge lists ---
    lists = [[None] * NB for _ in range(NCORES)]
    for c in range(NCORES):
        mc = np.nonzero(owner == c)[0]
        wsub = wid[mc]
        order = np.argsort(wsub, kind="stable")
        mc = mc[order]
        bounds = np.searchsorted(wsub[order], np.arange(NB + 1))
        for w in range(NB):
            m = mc[bounds[w]:bounds[w + 1]]
            m = m[np.argsort(g[m], kind="stable")]
            nlo = int(np.searchsorted(g[m], LO))
            lists[c][w] = (m[:nlo], m[nlo:])

    S_lo = [max(-(-len(lists[c][w][0]) // P) for c in range(NCORES))
            for w in range(NB)]
    S_hi = [max(-(-len(lists[c][w][1]) // P) for c in range(NCORES))
            for w in range(NB)]
    NSUBLO, NSUBHI = sum(S_lo), sum(S_hi)
    NSUB = NSUBLO + NSUBHI
    lo_pre = np.concatenate([[0], np.cumsum(S_lo)]).astype(int)
    hi_pre = np.concatenate([[0], np.cumsum(S_hi)]).astype(int)

    # schedule: per window, list of (stream, q) in consumption order
    sched = []
    for w in range(NB):
        subs = [("lo", lo_pre[w] + j) for j in range(S_lo[w])]
        subs += [("hi", hi_pre[w] + j) for j in range(S_hi[w])]
        sched.append(subs)

    # --- deg slot layout (own src rows) ---
    srcloc_all = src % RPC
    src_owner = src // RPC
    cnts = np.zeros((NCORES, RPC), np.int64)
    for c in range(NCORES):
        cnts[c] = np.bincount(srcloc_all[src_owner == c], minlength=RPC)
    K_DEG = int(cnts.max())

    # --- replicated x table (row-major, padded rows zero) ---
    xfull = np.zeros((NTOT, FIN), np.float32)
    for c in range(NCORES):
        xfull[c * RB:c * RB + RPC] = x[c * RPC:(c + 1) * RPC]

    iota_f = np.tile(np.arange(P, dtype=np.float32), (P, 1))
    ident_f = np.eye(P, dtype=np.float32)
    import ml_dtypes
    bf16 = ml_dtypes.bfloat16

    in_maps = []
    for c in range(NCORES):
        lo_idx = np.zeros(max(NSUBLO, 1) * P, np.int64)
        hi_idx = np.zeros(max(NSUBHI, 1) * P, np.int64)
        pair_idx = np.zeros(NSUB * P, np.int64)
        bcol = np.zeros((P, NSUB), np.float32)
        eacol = np.zeros((P, NSUB), np.float32)
        eacolA = np.zeros((P, NSUB), np.float32)
        eacolB = np.zeros((P, NSUB), np.float32)

        def fill(m, col):
            n = len(m)
            pair_idx[col * P:col * P + n] = g[m] // 2
            bcol[:n, col] = wloc[m]
            eacol[:n, col] = ea[m]
            par = (g[m] % 2).astype(np.float32)
            eacolA[:n, col] = ea[m] * (1.0 - par)
            eacolB[:n, col] = ea[m] * par

        for w in range(NB):
            elo, ehi = lists[c][w]
            for j in range(S_lo[w]):
                m = elo[j * P:(j + 1) * P]
                q = lo_pre[w] + j
                lo_idx[q * P:q * P + len(m)] = g[m]
                fill(m, q)
            for j in range(S_hi[w]):
                m = ehi[j * P:(j + 1) * P]
                q = hi_pre[w] + j
                hi_idx[q * P:q * P + len(m)] = g[m] - LO
                fill(m, NSUBLO + q)

        dslot = np.zeros((P, NB * K_DEG), np.float32)
        me = np.nonzero(src_owner == c)[0]
        slot_ctr = np.zeros(RPC, np.int64)
        locs = srcloc_all[me]
        for e, loc in zip(me, locs):
            s = slot_ctr[loc]
            slot_ctr[loc] += 1
            dslot[loc % P, (loc // P) * K_DEG + s] = ea[e]

        batchc = np.full((P, NB), 999.0, np.float32)
        blk = np.full(RB, 999.0, np.float32)
        blk[:RPC] = batch[c * RPC:(c + 1) * RPC]
        batchc[:, :] = blk.reshape(NB, P).T

        xT_own = np.zeros((FIN, RB), np.float32)
        xT_own[:, :RPC] = x[c * RPC:(c + 1) * RPC].T

        im = dict(
            xfull=xfull, xT=xT_own,
            glo=_wrap16(lo_idx), ghi=_wrap16(hi_idx),
            gpair=_wrap16(pair_idx),
            bcol=bcol, eacol=eacol, eacolA=eacolA, eacolB=eacolB,
            dslot=dslot, batchc=batchc,
            w1=Ws[0], w2=Ws[1], w3=Ws[2], wl=Wl,
            b1=bs[0][None, :], b2=bs[1][None, :], b3=bs[2][None, :],
            blb=np.tile(bl, (P, 1)),
            iotaf=iota_f, iotab=iota_f.astype(bf16),
            identb=ident_f.astype(bf16), identf=ident_f,
        )
        in_maps.append(im)

    hp = dict(NSUBLO=NSUBLO, NSUBHI=NSUBHI, NSUB=NSUB, K_DEG=K_DEG,
              S_lo=S_lo, S_hi=S_hi, sched=sched)
    return hp, in_maps


def _build(hp):
    import concourse.bacc as bacc
    import concourse.tile as tile
    import concourse.mybir as mybir
    from concourse import library_config
    dt = mybir.dt
    AF = mybir.ActivationFunctionType
    OP = mybir.AluOpType
    f32, bf = dt.float32, dt.bfloat16

    NSUBLO, NSUBHI, NSUB = hp["NSUBLO"], hp["NSUBHI"], hp["NSUB"]
    K_DEG, sched = hp["K_DEG"], hp["sched"]

    nc = bacc.Bacc("TRN2", target_bir_lowering=False, debug=False,
                   num_devices=NCORES, dynamic_dma_scratch_size=24576)

    xfull = nc.dram_tensor("xfull", [NTOT, FIN], f32, kind="ExternalInput")
    xT = nc.dram_tensor("xT", [FIN, RB], f32, kind="ExternalInput")
    glo = nc.dram_tensor("glo", [P, max(NSUBLO, 1) * 8], dt.int16,
                         kind="ExternalInput")
    ghi = nc.dram_tensor("ghi", [P, max(NSUBHI, 1) * 8], dt.int16,
                         kind="ExternalInput")
    gpair = nc.dram_tensor("gpair", [P, NSUB * 8], dt.int16,
                           kind="ExternalInput")
    bcol = nc.dram_tensor("bcol", [P, NSUB], f32, kind="ExternalInput")
    eacol = nc.dram_tensor("eacol", [P, NSUB], f32, kind="ExternalInput")
    eacolA = nc.dram_tensor("eacolA", [P, NSUB], f32, kind="ExternalInput")
    eacolB = nc.dram_tensor("eacolB", [P, NSUB], f32, kind="ExternalInput")
    dslot = nc.dram_tensor("dslot", [P, NB * K_DEG], f32,
                           kind="ExternalInput")
    batchc = nc.dram_tensor("batchc", [P, NB], f32, kind="ExternalInput")
    w1 = nc.dram_tensor("w1", [3, FIN, F1], f32, kind="ExternalInput")
    w2 = nc.dram_tensor("w2", [3, F1, F2], f32, kind="ExternalInput")
    w3 = nc.dram_tensor("w3", [3, F2, F3], f32, kind="ExternalInput")
    wl = nc.dram_tensor("wl", [F3, 2], f32, kind="ExternalInput")
    b1 = nc.dram_tensor("b1", [1, F1], f32, kind="ExternalInput")
    b2 = nc.dram_tensor("b2", [1, F2], f32, kind="ExternalInput")
    b3 = nc.dram_tensor("b3", [1, F3], f32, kind="ExternalInput")
    blb = nc.dram_tensor("blb", [P, 2], f32, kind="ExternalInput")
    iotaf = nc.dram_tensor("iotaf", [P, P], f32, kind="ExternalInput")
    iotab = nc.dram_tensor("iotab", [P, P], bf, kind="ExternalInput")
    identb = nc.dram_tensor("identb", [P, P], bf, kind="ExternalInput")
    identf = nc.dram_tensor("identf", [P, P], f32, kind="ExternalInput")
    y = nc.dram_tensor("y", [P, 2], f32, kind="ExternalOutput")

    with tile.TileContext(nc) as tc:
        with tc.tile_pool(name="cst", bufs=1) as cst, \
             tc.tile_pool(name="wk", bufs=4) as wk, \
             tc.tile_pool(name="wk1", bufs=1) as wk1, \
             tc.tile_pool(name="bp", bufs=6) as bp, \
             tc.tile_pool(name="slb", bufs=1) as slb, \
             tc.tile_pool(name="vlo", bufs=2) as vlo, \
             tc.tile_pool(name="vhi", bufs=2) as vhi, \
             tc.tile_pool(name="psm", bufs=2, space="PSUM") as psm, \
             tc.tile_pool(name="psz", bufs=2, space="PSUM") as psz, \
             tc.tile_pool(name="pstr", bufs=1, space="PSUM") as pstr, \
             tc.tile_pool(name="psfin", bufs=1, space="PSUM") as psfin, \
             tc.tile_pool(name="dram", bufs=1, space="DRAM") as dram:

            nc.gpsimd.load_library(library_config.mlp)

            # ---------------- dram tables ----------------
            Tx = dram.tile([NTOT, XW], bf, tag="Tx", name="Tx")
            Tz1 = dram.tile([NTOT, TW], bf, tag="Tz1", name="Tz1",
                            addr_space="Shared")
            Th1 = dram.tile([NTOT, TW], bf, tag="Th1", name="Th1",
                            addr_space="Shared")
            Tz2 = dram.tile([NTOT // 2, TW], bf, tag="Tz2", name="Tz2",
                            addr_space="Shared")
            Th2 = dram.tile([NTOT // 2, TW], bf, tag="Th2", name="Th2",
                            addr_space="Shared")
            Tz3 = dram.tile([NTOT // 2, TW], bf, tag="Tz3", name="Tz3",
                            addr_space="Shared")
            zcon = dram.tile([RB, TW], bf, tag="zcon", name="zcon")
            zcon2 = dram.tile([RB, TW // 2], bf, tag="zcon2", name="zcon2")
            hcon = dram.tile([RB, TW], bf, tag="hcon", name="hcon")
            hcon2 = dram.tile([RB, TW // 2], bf, tag="hcon2", name="hcon2")
            hown = dram.tile([RB, TW], bf, tag="hown", name="hown")
            xTbf = dram.tile([FIN, RB], bf, tag="xTbf", name="xTbf")
            degsh = dram.tile([RB, 1], f32, tag="degsh", name="degsh")
            degf = dram.tile([NTOT, 1], f32, tag="degf", name="degf")
            arin = dram.tile([P, F3 + 1], f32, tag="arin", name="arin")
            arout = dram.tile([P, F3 + 1], f32, tag="arout", name="arout")

            # ---------------- consts ----------------
            iotab_t = cst.tile([P, P], bf)
            nc.sync.dma_start(out=iotab_t[:], in_=iotab[:, :])
            iotaf_t = cst.tile([P, P], f32)
            nc.sync.dma_start(out=iotaf_t[:], in_=iotaf[:, :])
            identb_t = cst.tile([P, P], bf)
            nc.sync.dma_start(out=identb_t[:], in_=identb[:, :])
            identf_t = cst.tile([P, P], f32)
            nc.sync.dma_start(out=identf_t[:], in_=identf[:, :])
            bcol_t = cst.tile([P, NSUB], f32)
            nc.sync.dma_start(out=bcol_t[:], in_=bcol[:, :])
            eacol_t = cst.tile([P, NSUB], f32)
            nc.sync.dma_start(out=eacol_t[:], in_=eacol[:, :])
            eaA_t = cst.tile([P, NSUB], f32)
            nc.sync.dma_start(out=eaA_t[:], in_=eacolA[:, :])
            eaB_t = cst.tile([P, NSUB], f32)
            nc.sync.dma_start(out=eaB_t[:], in_=eacolB[:, :])
            glo_t = cst.tile([P, max(NSUBLO, 1) * 8], dt.int16)
            nc.sync.dma_start(out=glo_t[:], in_=glo[:, :])
            ghi_t = cst.tile([P, max(NSUBHI, 1) * 8], dt.int16)
            nc.sync.dma_start(out=ghi_t[:], in_=ghi[:, :])
            gpair_t = cst.tile([P, NSUB * 8], dt.int16)
            nc.sync.dma_start(out=gpair_t[:], in_=gpair[:, :])
            batchc_t = cst.tile([P, NB], f32)
            nc.sync.dma_start(out=batchc_t[:], in_=batchc[:, :])
            ones1 = cst.tile([1, P], f32)
            nc.vector.memset(ones1[:], 1.0)
            blt = cst.tile([P, 2], f32)
            nc.sync.dma_start(out=blt[:], in_=blb[:, :])
            wlt = cst.tile([P, 2], f32)
            nc.sync.dma_start(out=wlt[:F3, :], in_=wl[:, :])
            b_t = []
            for bb, fo in ((b1, F1), (b2, F2), (b3, F3)):
                t = cst.tile([1, fo], f32, tag=f"b{fo}")
                nc.sync.dma_start(out=t[:], in_=bb[:, :])
                b_t.append(t)

            # weights -> bf16 chunk tiles: Wa = W0 - W2, Wb = W1, Wc = W2
            layer_w = []
            for li, (wt_, fin, fo) in enumerate(
                    ((w1, FIN, F1), (w2, F1, F2), (w3, F2, F3))):
                nch = (fin + P - 1) // P
                was, wbs, wcs = [], [], []
                for o in range(nch):
                    kp = min(P, fin - o * P)
                    t0 = wk.tile([P, fo], f32, tag="wld", bufs=2)
                    nc.sync.dma_start(out=t0[:kp, :],
                                      in_=wt_[0, o * P:o * P + kp, :])
                    t2 = wk.tile([P, fo], f32, tag="wld", bufs=2)
                    nc.sync.dma_start(out=t2[:kp, :],
                                      in_=wt_[2, o * P:o * P + kp, :])
                    t1 = wk.tile([P, fo], f32, tag="wld", bufs=2)
                    nc.sync.dma_start(out=t1[:kp, :],
                                      in_=wt_[1, o * P:o * P + kp, :])
                    wa = cst.tile([P, fo], bf, tag=f"wa{li}_{o}")
                    nc.vector.tensor_tensor(out=wa[:kp, :], in0=t0[:kp, :],
                                            in1=t2[:kp, :], op=OP.subtract)
                    wb_ = cst.tile([P, fo], bf, tag=f"wb{li}_{o}")
                    nc.vector.tensor_copy(out=wb_[:kp, :], in_=t1[:kp, :])
                    wc_ = cst.tile([P, fo], bf, tag=f"wc{li}_{o}")
                    nc.vector.tensor_copy(out=wc_[:kp, :], in_=t2[:kp, :])
                    was.append((wa, kp))
                    wbs.append((wb_, kp))
                    wcs.append((wc_, kp))
                layer_w.append(dict(wa=was, wb=wbs, wc=wcs))

            # xT -> bf16 DRAM copy (lhsT source for l=0 dense mms)
            XCH = [(0, P), (1, FIN - P)]
            for o, st_tag in ((0, "hslab"), (1, "htslab")):
                kp = min(P, FIN - o * P)
                xstage = slb.tile([P, RB], bf, tag=st_tag,
                                  name=f"xstage{o}")
                nc.gpsimd.dma_start(out=xstage[:kp, :],
                                    in_=xT[o * P:o * P + kp, :])
                nc.sync.dma_start(out=xTbf[o * P:o * P + kp, :],
                                  in_=xstage[:kp, :])

            # ---------------- deg / dinv ----------------
            degsb = wk1.tile([P, NB], f32, tag="degsb")
            CH_D = 7
            for c0 in range(0, NB, CH_D):
                ch = min(CH_D, NB - c0)
                t = wk.tile([P, CH_D * K_DEG], f32, tag="dgl", bufs=2)
                nc.sync.dma_start(
                    out=t[:, :ch * K_DEG],
                    in_=dslot[:, c0 * K_DEG:(c0 + ch) * K_DEG])
                nc.vector.tensor_reduce(
                    out=degsb[:, c0:c0 + ch, None],
                    in_=t[:, :ch * K_DEG].rearrange("p (b k) -> p b k",
                                                    k=K_DEG),
                    axis=mybir.AxisListType.X, op=OP.add)
            nc.sync.dma_start(
                out=degsh[:].rearrange("(b p) c -> p (b c)", p=P),
                in_=degsb[:])
            nc.gpsimd.collective_compute(
                "AllGather", OP.bypass, replica_groups=[list(range(NCORES))],
                ins=[degsh[:, :].opt()], outs=[degf[:, :].opt()])

            def dinv_of(deg_ap, cols, tag):
                m = wk1.tile([P, cols], f32, tag=tag + "m")
                nc.vector.tensor_scalar(out=m[:], in0=deg_ap, scalar1=0.0,
                                        scalar2=None, op0=OP.is_le)
                safe = wk1.tile([P, cols], f32, tag=tag + "s")
                nc.vector.tensor_tensor(out=safe[:], in0=deg_ap, in1=m[:],
                                        op=OP.add)
                sq = wk1.tile([P, cols], f32, tag=tag + "q")
                nc.scalar.activation(out=sq[:], in_=safe[:], func=AF.Sqrt)
                rcp = wk1.tile([P, cols], f32, tag=tag + "r")
                nc.vector.reciprocal(rcp[:], sq[:])
                gm = wk1.tile([P, cols], f32, tag=tag + "g")
                nc.vector.tensor_scalar(out=gm[:], in0=deg_ap, scalar1=0.0,
                                        scalar2=None, op0=OP.is_gt)
                dv = cst.tile([P, cols], f32, tag=tag + "d")
                nc.vector.tensor_tensor(out=dv[:], in0=rcp[:], in1=gm[:],
                                        op=OP.mult)
                return dv

            dinv_own = dinv_of(degsb[:], NB, "dow")
            negd_own = cst.tile([P, NB], f32)
            nc.vector.tensor_scalar_mul(negd_own[:], dinv_own[:], -1.0)
            d2 = wk1.tile([P, NB], f32, tag="d2")
            nc.vector.tensor_tensor(out=d2[:], in0=dinv_own[:],
                                    in1=dinv_own[:], op=OP.mult)
            neg2d2_own = cst.tile([P, NB], f32)
            nc.vector.tensor_scalar_mul(neg2d2_own[:], d2[:], -2.0)

            NCOLT = NTOT // P
            degfsb = wk1.tile([P, NCOLT], f32, tag="degfsb")
            nc.sync.dma_start(
                out=degfsb[:],
                in_=degf[:, 0:1].rearrange("(b p) c -> p (b c)", p=P))
            dinv_full = dinv_of(degfsb[:], NCOLT, "dfu")

            # ---------------- x~ table build ----------------
            xf_v = xfull[:, :].rearrange("(t p) f -> p t f", p=P)
            tx_v = Tx[:].rearrange("(t p) f -> p t f", p=P)
            CH_X = 14
            for t0 in range(0, NCOLT, CH_X):
                ch = min(CH_X, NCOLT - t0)
                xt_ = wk.tile([P, CH_X, FIN], f32, tag="xld", bufs=2)
                nc.sync.dma_start(out=xt_[:, :ch, :],
                                  in_=xf_v[:, t0:t0 + ch, :])
                xs = wk.tile([P, CH_X, FIN], bf, tag="xsc", bufs=2)
                for t in range(ch):
                    nc.scalar.activation(
                        out=xs[:, t, :], in_=xt_[:, t, :], func=AF.Copy,
                        scale=dinv_full[:, t0 + t:t0 + t + 1])
                nc.sync.dma_start(out=tx_v[:, t0:t0 + ch, 0:FIN],
                                  in_=xs[:, :ch, :])

            # ---------------- propagate machinery ----------------
            def run_propagate(streams, twidth, realw, call_sg, win_fn,
                              paired):
                """streams: {name: (idx_tile, sub_offset, nsub, table_ap,
                pool)}; gathers + B-matmul scatter over all windows."""
                state = {}
                for stname, (gt, goff, nsub_s, tap, pool) in streams.items():
                    ncalls = -(-nsub_s // call_sg) if nsub_s else 0
                    state[stname] = dict(gt=gt, goff=goff, nsub=nsub_s,
                                         tap=tap, pool=pool, issued=0,
                                         tiles=[], ncalls=ncalls)

                def issue(st):
                    s = state[st]
                    a = s["issued"] * call_sg
                    b = min(a + call_sg, s["nsub"])
                    nsg = b - a
                    vt = s["pool"].tile([P, call_sg, twidth], bf,
                                        tag=f"v{st}", name=f"v{st}")
                    nc.gpsimd.dma_gather(
                        out_ap=vt[:, :nsg, :], in_ap=s["tap"],
                        idxs_ap=s["gt"][:, (s["goff"] + a) * 8:
                                        (s["goff"] + b) * 8],
                        num_idxs=nsg * P, num_idxs_reg=nsg * P,
                        elem_size=twidth, single_packet=False)
                    s["tiles"].append((vt, a))
                    s["issued"] += 1

                def get(st, q):
                    s = state[st]
                    while s["issued"] * call_sg <= q:
                        issue(st)
                    if s["issued"] < s["ncalls"] and \
                            q >= (s["issued"] - 1) * call_sg + call_sg // 2:
                        issue(st)
                    ci = q // call_sg
                    vt, a = s["tiles"][ci]
                    return vt, q - a

                for w in range(NB):
                    subs = sched[w]
                    psm_t = None
                    if subs:
                        psm_t = psm.tile([P, realw], f32, tag="psm",
                                         name="psm_t")
                        nmm = len(subs) * (2 if paired else 1)
                        i = 0
                        for st, q in subs:
                            col = q if st == "lo" else NSUBLO + q
                            vt, slot = get(st, q)
                            if paired:
                                for eat, off in ((eaA_t, 0), (eaB_t, 64)):
                                    B = bp.tile([P, P], bf, tag="B",
                                                name="B")
                                    nc.vector.tensor_scalar(
                                        out=B[:], in0=iotab_t[:],
                                        scalar1=bcol_t[:, col:col + 1],
                                        scalar2=eat[:, col:col + 1],
                                        op0=OP.is_equal, op1=OP.mult)
                                    nc.tensor.matmul(
                                        psm_t[:], B[:],
                                        vt[:, slot, off:off + realw],
                                        start=(i == 0), stop=(i == nmm - 1))
                                    i += 1
                            else:
                                B = bp.tile([P, P], bf, tag="B", name="B")
                                nc.vector.tensor_scalar(
                                    out=B[:], in0=iotab_t[:],
                                    scalar1=bcol_t[:, col:col + 1],
                                    scalar2=eacol_t[:, col:col + 1],
                                    op0=OP.is_equal, op1=OP.mult)
                                nc.tensor.matmul(
                                    psm_t[:], B[:], vt[:, slot, 0:realw],
                                    start=(i == 0), stop=(i == nmm - 1))
                                i += 1
                    win_fn(w, psm_t)

            def unpaired_streams(tbl):
                return {"lo": (glo_t, 0, NSUBLO, tbl[0:LO, :], vlo),
                        "hi": (ghi_t, 0, NSUBHI, tbl[LO:NTOT, :], vhi)}

            def paired_streams(tbl):
                return {"lo": (gpair_t, 0, NSUBLO, tbl[:, :], vlo),
                        "hi": (gpair_t, NSUBLO, NSUBHI, tbl[:, :], vhi)}

            # ---------------- layers ----------------
            layer_cfg = [
                dict(fin=FIN, fout=F1, p1_paired=False, p2_paired=False,
                     tin=Tx, tin_w=XW, tz=Tz1, th=Th1),
                dict(fin=F1, fout=F2, p1_paired=False, p2_paired=True,
                     tin=Th1, tin_w=TW, tz=Tz2, th=Th2),
                dict(fin=F2, fout=F3, p1_paired=True, p2_paired=True,
                     tin=Th2, tin_w=TW, tz=Tz3, th=None),
            ]

            for li, cfg in enumerate(layer_cfg):
                fin, fout = cfg["fin"], cfg["fout"]
                lw = layer_w[li]
                nch_in = (fin + P - 1) // P

                # ---- P1 pre-pass: zpre[w] = (H~ W1)[own w] ----
                zpre = slb.tile([P, NB, F1], bf, tag="pre", name="zpre")
                for w in range(NB):
                    psz1 = psz.tile([P, fout], f32, tag="psz", name="psz1")
                    if li == 0:
                        for o, kp in XCH:
                            xw = wk.tile([P, P], bf, tag="xw")
                            nc.sync.dma_start(
                                out=xw[:kp, :],
                                in_=xTbf[o * P:o * P + kp,
                                         w * P:(w + 1) * P])
                            nc.tensor.matmul(
                                psz1[:], xw[:kp, :], lw["wb"][o][0][:kp, :],
                                start=(o == 0), stop=(o == len(XCH) - 1))
                    else:
                        for o in range(nch_in):
                            ht = wk.tile([P, P], bf, tag="htT")
                            nc.sync.dma_start(
                                out=ht[:],
                                in_=hcon[w * P:(w + 1) * P,
                                         o * P:(o + 1) * P],
                                transpose=True)
                            kp = lw["wb"][o][1]
                            nc.tensor.matmul(
                                psz1[:], ht[:kp, :], lw["wb"][o][0][:kp, :],
                                start=(o == 0), stop=(o == nch_in - 1))
                    if li == 0:
                        nc.scalar.activation(
                            out=zpre[:, w, 0:fout], in_=psz1[:],
                            func=AF.Copy, scale=dinv_own[:, w:w + 1])
                    else:
                        nc.scalar.activation(out=zpre[:, w, 0:fout],
                                             in_=psz1[:], func=AF.Copy)

                # ---- P1: M1 -> Z~ own ----
                zslab = slb.tile([P, NB, F1], bf, tag="zslab", name="zslab")

                def p1_win(w, psm_t, lw=lw, fin=fin, fout=fout,
                           nch_in=nch_in, zpre=zpre, zslab=zslab):
                    if psm_t is None:
                        nc.vector.tensor_copy(out=zslab[:, w, 0:fout],
                                              in_=zpre[:, w, 0:fout])
                        return
                    mt = wk.tile([P, fin], bf, tag="mt")
                    nc.scalar.activation(
                        out=mt[:], in_=psm_t[:, 0:fin], func=AF.Copy,
                        scale=neg2d2_own[:, w:w + 1])
                    psz2 = psz.tile([P, fout], f32, tag="psz", name="psz2")
                    for o in range(nch_in):
                        kp = min(P, fin - o * P)
                        pt = pstr.tile([P, P], bf, tag="pt")
                        nc.tensor.transpose(
                            out=pt[:kp, :], in_=mt[:, o * P:o * P + kp],
                            identity=identb_t[:])
                        mtt = wk.tile([P, P], bf, tag="mtt")
                        nc.scalar.activation(out=mtt[:kp, :], in_=pt[:kp, :],
                                             func=AF.Copy)
                        nc.tensor.matmul(
                            psz2[:], mtt[:kp, :], lw["wc"][o][0][:kp, :],
                            start=(o == 0), stop=(o == nch_in - 1))
                    nc.vector.tensor_tensor(
                        out=zslab[:, w, 0:fout], in0=psz2[:],
                        in1=zpre[:, w, 0:fout], op=OP.add)

                run_propagate(
                    paired_streams(cfg["tin"]) if cfg["p1_paired"]
                    else unpaired_streams(cfg["tin"]),
                    cfg["tin_w"], fin,
                    CALL_SG_X if li == 0 else CALL_SG_T, p1_win,
                    cfg["p1_paired"])

                if cfg["p2_paired"]:
                    # packed contribution: [RB, 64] (real fout cols)
                    nc.sync.dma_start(
                        out=zcon2[:, :].rearrange("(w p) f -> p w f", p=P)
                        [:, :, 0:fout],
                        in_=zslab[:, :, 0:fout])
                    nc.gpsimd.collective_compute(
                        "AllGather", OP.bypass,
                        replica_groups=[list(range(NCORES))],
                        ins=[zcon2[:, :].opt()], outs=[cfg["tz"][:].opt()])
                else:
                    nc.sync.dma_start(
                        out=zcon[:, :].rearrange("(w p) f -> p w f", p=P)
                        [:, :, 0:fout],
                        in_=zslab[:, :, 0:fout])
                    nc.gpsimd.collective_compute(
                        "AllGather", OP.bypass,
                        replica_groups=[list(range(NCORES))],
                        ins=[zcon[:, :].opt()], outs=[cfg["tz"][:].opt()])

                # ---- P2 pre-pass: dpre[w] = (H Wa + b)[own w] ----
                dpre = slb.tile([P, NB, F1], bf, tag="pre", name="dpre")
                for w in range(NB):
                    psd_t = psz.tile([P, fout], f32, tag="psz", name="psd_t")
                    if li == 0:
                        for o, kp in XCH:
                            xw = wk.tile([P, P], bf, tag="xw")
                            nc.sync.dma_start(
                                out=xw[:kp, :],
                                in_=xTbf[o * P:o * P + kp,
                                         w * P:(w + 1) * P])
                            nc.tensor.matmul(
                                psd_t[:], xw[:kp, :], lw["wa"][o][0][:kp, :],
                                start=(o == 0), stop=False)
                    else:
                        for o in range(nch_in):
                            ht = wk.tile([P, P], bf, tag="hoT")
                            nc.sync.dma_start(
                                out=ht[:],
                                in_=hown[w * P:(w + 1) * P, 0:P],
                                transpose=True)
                            kp = lw["wa"][o][1]
                            nc.tensor.matmul(
                                psd_t[:], ht[:kp, :], lw["wa"][o][0][:kp, :],
                                start=(o == 0), stop=False)
                    nc.tensor.matmul(psd_t[:], ones1[:, :],
                                     b_t[li][:, :], start=False, stop=True)
                    nc.scalar.activation(out=dpre[:, w, 0:fout],
                                         in_=psd_t[:], func=AF.Copy)

                # ---- P2: M2 -> H' ----
                hslab = htslab = pooled = None
                if li < 2:
                    hslab = slb.tile([P, NB, F1], bf, tag="hslab",
                                     name="hslab")
                    htslab = slb.tile([P, NB, F1], bf, tag="htslab",
                                      name="htslab")
                else:
                    pooled = psfin.tile([P, F3 + 1], f32, tag="pooled",
                                        name="pooled")

                def p2_win(w, psm_t, li=li, fout=fout, dpre=dpre,
                           hslab=hslab, htslab=htslab, pooled=pooled):
                    if psm_t is not None:
                        v2 = wk.tile([P, fout], f32, tag="v2")
                        nc.scalar.activation(
                            out=v2[:], in_=psm_t[:, 0:fout], func=AF.Copy,
                            scale=negd_own[:, w:w + 1])
                        s = wk.tile([P, fout], f32, tag="s")
                        nc.vector.tensor_tensor(
                            out=s[:], in0=v2[:], in1=dpre[:, w, 0:fout],
                            op=OP.add)
                        src_ap = s[:]
                    else:
                        src_ap = dpre[:, w, 0:fout]
                    if li < 2:
                        nc.scalar.activation(out=hslab[:, w, 0:fout],
                                             in_=src_ap, func=AF.Relu)
                        nc.scalar.activation(
                            out=htslab[:, w, 0:fout], in_=src_ap,
                            func=AF.Relu, scale=dinv_own[:, w:w + 1])
                    else:
                        r33 = wk.tile([P, F3 + 1], f32, tag="r33")
                        nc.vector.memset(r33[:], 1.0)
                        nc.scalar.activation(out=r33[:, 0:F3], in_=src_ap,
                                             func=AF.Relu)
                        Bp = wk.tile([P, P], f32, tag="Bp")
                        nc.vector.tensor_scalar(
                            out=Bp[:], in0=iotaf_t[:],
                            scalar1=batchc_t[:, w:w + 1], scalar2=None,
                            op0=OP.is_equal)
                        nc.tensor.matmul(pooled[:], Bp[:], r33[:],
                                         start=(w == 0), stop=(w == NB - 1))

                run_propagate(
                    paired_streams(cfg["tz"]) if cfg["p2_paired"]
                    else unpaired_streams(cfg["tz"]),
                    TW, fout, CALL_SG_T, p2_win, cfg["p2_paired"])

                if li < 2:
                    nc.sync.dma_start(
                        out=hown[:, :].rearrange("(w p) f -> p w f", p=P)
                        [:, :, 0:fout],
                        in_=hslab[:, :, 0:fout])
                    if li == 0:
                        nc.sync.dma_start(
                            out=hcon[:, :].rearrange("(w p) f -> p w f",
                                                     p=P)[:, :, 0:fout],
                            in_=htslab[:, :, 0:fout])
                        nc.gpsimd.collective_compute(
                            "AllGather", OP.bypass,
                            replica_groups=[list(range(NCORES))],
                            ins=[hcon[:, :].opt()],
                            outs=[cfg["th"][:].opt()])
                    else:
                        # local full-width staging for next P1's transposes
                        nc.sync.dma_start(
                            out=hcon[:, :].rearrange("(w p) f -> p w f",
                                                     p=P)[:, :, 0:fout],
                            in_=htslab[:, :, 0:fout])
                        nc.sync.dma_start(
                            out=hcon2[:, :].rearrange("(w p) f -> p w f",
                                                      p=P)[:, :, 0:fout],
                            in_=htslab[:, :, 0:fout])
                        nc.gpsimd.collective_compute(
                            "AllGather", OP.bypass,
                            replica_groups=[list(range(NCORES))],
                            ins=[hcon2[:, :].opt()],
                            outs=[cfg["th"][:].opt()])

            # ---------------- pooled mean + head ----------------
            psb = wk1.tile([P, F3 + 1], f32, tag="psb")
            nc.vector.tensor_copy(out=psb[:], in_=pooled[:])
            nc.sync.dma_start(out=arin[:, :], in_=psb[:])
            nc.gpsimd.collective_compute(
                "AllReduce", OP.add, replica_groups=[list(range(NCORES))],
                ins=[arin[:, :].opt()], outs=[arout[:, :].opt()])
            pr = wk1.tile([P, F3 + 1], f32, tag="pr")
            nc.sync.dma_start(out=pr[:], in_=arout[:, :])
            cmax = wk1.tile([P, 1], f32, tag="cmax")
            nc.vector.tensor_scalar_max(cmax[:], pr[:, F3:F3 + 1], 1.0)
            rcp = wk1.tile([P, 1], f32, tag="rcpf")
            nc.vector.reciprocal(rcp[:], cmax[:])
            pm = wk1.tile([P, F3], f32, tag="pm")
            nc.scalar.activation(out=pm[:], in_=pr[:, 0:F3], func=AF.Copy,
                                 scale=rcp[:, 0:1])
            ptp = pstr.tile([P, P], f32, tag="ptr")
            nc.tensor.transpose(out=ptp[:F3, :], in_=pm[:],
                                identity=identf_t[:])
            pmT = wk1.tile([P, P], f32, tag="pmT")
            nc.scalar.activation(out=pmT[:F3, :], in_=ptp[:F3, :],
                                 func=AF.Copy)
            psy = psfin.tile([P, 2], f32, tag="psy")
            nc.tensor.matmul(psy[:], pmT[:F3, :], wlt[:F3, :], start=True,
                             stop=True)
            yt = wk1.tile([P, 2], f32, tag="yt")
            nc.vector.tensor_tensor(out=yt[:], in0=psy[:], in1=blt[:],
                                    op=OP.add)
            nc.sync.dma_start(out=y[:, :], in_=yt[:])

    nc.compile()
    return nc


def kernel(**inputs):
    hp, in_maps = _prep(inputs)
    nc = _build(hp)
    from concourse import bass_utils
    res = bass_utils.run_bass_kernel_spmd(nc, in_maps,
                                          core_ids=list(range(NCORES)))
    return np.asarray(res.results[0]["y"], np.float32)
